# revision 27
# baseline (speedup 1.0000x reference)
"""Trainium2 Bass kernel for NeuralGraphHidden (GNN message passing).

Full-input contract: kernel(**inputs) takes the complete unsharded arrays,
shards batch dim 0 across 8 NeuronCores (data parallel), runs one SPMD Bass
program, and reassembles the full output.

Key structural fact exploited: deg[a] = #(edges[a,:] != -1) is in 0..5, but
the reference's degree mask covers only 0..4 - atoms with deg==5 (about 96%
of atoms for this input distribution) produce an all-zero output row.  The
kernel compacts the few deg<5 atoms per molecule into static per-degree
slots on-chip and runs the dense layer only on those slots:

  per group of 8 molecules (4 groups/core, 32 molecules/core):
    sumbond   = bonds pre-summed over the 5 bond slots by the
                DMA compute engine (accum_op=add)             (DMA CCE)
    deg       = row counts of edges != -1                     (DVE)
    rank_d    = per-degree prefix sums via tri-matmul          (PE)
    P         = slot one-hot (atom -> slot), 32-padded         (DVE)
    gatt      = per-slot edge ids via edeg^T @ P               (PE)
    E_j       = edge ids broadcast down partitions             (PE)
    ET        = sum_j onehot(E_j) + P  (neighbour+self)        (DVE)
    nstt|braw = [atoms | sumbond] gathered per slot            (PE)
    f0/f1/f2  = feats re-permuted slot->degree-block via
                constant permutation matmuls                   (PE)
    z4/z3     = per-degree dense with W_4 / W_3 (+bias row)    (PE)
    out       = relu(z)                                        (ScalarE)

  Matmul PSUM outputs may only start at partition 0/32/64, so per-molecule
  slot rows are 32-padded and grouped 3+3+2 molecules into three gather
  tiles; the dense stage uses two tiles (deg4: 96 rows, deg3: 32 rows).
  Slot capacities: 12 deg-4 + 4 deg-3 per molecule (measured maxima 12/2).
  deg<=2 atoms and any capacity overflow fall back to a tiny numpy path on
  the host (0-1 atoms in practice).  The host scatters the compact HW rows
  into the zero-initialised full output.
"""

import sys

sys.path.insert(0, "/opt/trn_rl_repo")

import numpy as np

B, A, D = 256, 128, 5
FA, FB, C = 256, 64, 256
NCORES = 8
BL = B // NCORES   # 32 molecules per core
G = 8              # molecules per group
NG = BL // G       # 4 groups per core

SJ = 32            # padded slots per molecule (16 real: 12 deg4 + 4 deg3)
SR = 16            # real slots per molecule
CAP4, CAP3 = 12, 4
Q = G * SR         # 128 dense-stage slots per group
Q3_BASE = G * CAP4  # 96: deg3 block starts here in q space
TILES3 = ((0, 3), (3, 3), (6, 2))   # (first molecule, count) per gather tile

# packed-const column offsets (cpack, bf16, 128 partitions)
C_LTRI = 0          # (128, 128)
C_IOTAJ = 128       # (128, 256)
C_LADDER = 384      # (128, 16)
C_IOTAC = 400       # (128, 1)
C_SEL = 401         # (5, 640)
C_TOT = 1041

_CACHE = {}


def _build_program():
    from contextlib import ExitStack

    import concourse.bass as bass
    import concourse.tile as tile
    from concourse import bacc, mybir

    f32 = mybir.dt.float32
    bf16 = mybir.dt.bfloat16
    AF = mybir.ActivationFunctionType
    OP = mybir.AluOpType
    AX = mybir.AxisListType

    nc = bacc.Bacc("TRN2", target_bir_lowering=False, debug=False,
                   num_devices=NCORES)

    atoms_d = nc.dram_tensor("atoms", [NG, A, G * FA], bf16,
                             kind="ExternalInput")
    bonds_d = nc.dram_tensor("bonds", [NG, A, G * D * FB], bf16,
                             kind="ExternalInput")
    edeg_d = nc.dram_tensor("edeg", [NG, A, G * D], bf16,
                            kind="ExternalInput")
    cpack_d = nc.dram_tensor("cpack", [A, C_TOT], bf16, kind="ExternalInput")
    iotac_d = nc.dram_tensor("iotac", [A, 1], f32, kind="ExternalInput")
    rpack_d = nc.dram_tensor("rpack", [96, 3 * Q], bf16,
                             kind="ExternalInput")
    wpack_d = nc.dram_tensor("wpack", [128, 3 * 2 * C], bf16,
                             kind="ExternalInput")
    out_d = nc.dram_tensor("out", [NG, Q, C], bf16, kind="ExternalOutput")

    atoms_ap = atoms_d.ap()
    bonds_ap = bonds_d.ap()
    edeg_ap = edeg_d.ap()
    out_ap = out_d.ap()

    with tile.TileContext(nc) as tc, ExitStack() as ctx:
        consts = ctx.enter_context(tc.tile_pool(name="consts", bufs=1))
        pin = ctx.enter_context(tc.tile_pool(name="pin", bufs=2))
        pmid = ctx.enter_context(tc.tile_pool(name="pmid", bufs=2))
        pout = ctx.enter_context(tc.tile_pool(name="pout", bufs=2))
        ps_e = ctx.enter_context(
            tc.tile_pool(name="ps_e", bufs=1, space="PSUM"))
        ps_g = ctx.enter_context(
            tc.tile_pool(name="ps_g", bufs=1, space="PSUM"))

        # ---- packed constants: 3 DMAs total -----------------------------
        cpack = consts.tile([A, C_TOT], bf16)
        nc.scalar.dma_start(out=cpack[:], in_=cpack_d.ap()[:])
        rpack = consts.tile([96, 3 * Q], bf16)
        nc.scalar.dma_start(out=rpack[:], in_=rpack_d.ap()[:])
        wpack = consts.tile([128, 3 * 2 * C], bf16)
        nc.scalar.dma_start(out=wpack[:], in_=wpack_d.ap()[:])
        iotac_t = consts.tile([A, 1], f32)
        nc.scalar.dma_start(out=iotac_t[:], in_=iotac_d.ap()[:])

        ltri = cpack[:, C_LTRI:C_LTRI + 128]
        iotaj = cpack[:, C_IOTAJ:C_IOTAJ + G * SJ]
        ladder = cpack[:, C_LADDER:C_LADDER + 2 * G]
        iotac = iotac_t[:]
        sel = cpack[0:5, C_SEL:C_SEL + 5 * A]
        rts = [rpack[0:96, 0:Q], rpack[0:96, Q:2 * Q],
               rpack[0:64, 2 * Q:3 * Q]]
        w0 = wpack[:, 0:2 * C]
        w1 = wpack[:, 2 * C:4 * C]
        w2 = wpack[0:FB + 1, 4 * C:6 * C]

        for bg in range(NG):
            # ---- inputs; bonds pre-summed over d by DMA CCE -------------
            edeg_g = pin.tile([A, G * D], bf16)
            nc.sync.dma_start(out=edeg_g[:], in_=edeg_ap[bg])
            atoms_g = pin.tile([A, G * FA], bf16)
            nc.sync.dma_start(out=atoms_g[:], in_=atoms_ap[bg])
            sb_g = pin.tile([A, G * FB], bf16)
            bview = bonds_ap[bg].rearrange("p (g d f) -> p g d f", g=G, d=D)
            sview = sb_g.rearrange("p (g f) -> p g f", g=G)
            nc.sync.dma_start(out=sview, in_=bview[:, :, 0, :])
            with nc.allow_low_precision(reason="bf16 bond sums via CCE"):
                for k in (1, 2, 3, 4):
                    nc.gpsimd.dma_start(out=sview, in_=bview[:, :, k, :],
                                        accum_op=OP.add)

            # PSUM bank packing (8 banks of 2KB):
            #   small1 x2, e04 x2, misc, z3t, gt x2
            small1 = ps_e.tile([A, 400], f32, tag="small1", bufs=2)
            pos2 = small1[:, 0:16]
            gatt = small1[0:5, 16:144]
            f0p = small1[:, 144:272]
            e4 = small1[:, 272:400]
            e04 = ps_e.tile([A, 4 * Q], f32, tag="e04", bufs=2)
            misc = ps_e.tile([A, 384], f32, tag="misc", bufs=1)
            f2p = misc[0:FB, 0:Q]
            z4 = misc[0:Q3_BASE, Q:Q + C]
            z3t = ps_e.tile([A, 384], f32, tag="z3t", bufs=1)
            z3 = z3t[0:Q - Q3_BASE, 0:C]
            f1p = z3t[:, C:C + Q]

            # ---- degree + per-degree ranks ------------------------------
            ne = pmid.tile([A, G * D], bf16)
            nc.vector.tensor_scalar(ne[:], edeg_g[:], -1.0, None,
                                    OP.not_equal)
            deg = pmid.tile([A, G], bf16)
            with nc.allow_low_precision(reason="counts <= 5 exact in bf16"):
                nc.vector.tensor_reduce(
                    deg[:], ne.rearrange("p (g d) -> p g d", g=G),
                    axis=AX.X, op=OP.add)
            masks2 = pmid.tile([A, 2 * G], bf16)
            nc.vector.tensor_tensor(
                masks2.rearrange("p (d g) -> p d g", d=2),
                ladder.rearrange("p (d g) -> p d g", d=2),
                deg.unsqueeze(1).broadcast_to((A, 2, G)),
                OP.is_equal)
            nc.tensor.matmul(pos2, ltri, masks2[:])
            posm2 = pmid.tile([A, 2 * G], bf16)
            with nc.allow_low_precision(reason="ranks <= 128 exact in bf16"):
                nc.vector.tensor_tensor(posm2[:], pos2, masks2[:], OP.mult)

            # ---- slot one-hot P, 32-padded per molecule -----------------
            pmm = pmid.tile([A, G * SJ], bf16)
            nc.gpsimd.memset(pmm[:], 0.0)
            pv = pmm.rearrange("p (m j) -> p m j", m=G)
            iv = iotaj.rearrange("p (m j) -> p m j", m=G)
            nc.vector.tensor_tensor(
                pv[:, :, 0:CAP4], iv[:, :, 0:CAP4],
                posm2[:, G:2 * G].unsqueeze(2).broadcast_to((A, G, CAP4)),
                OP.is_equal)
            nc.vector.tensor_tensor(
                pv[:, :, CAP4:SR], iv[:, :, CAP4:SR],
                posm2[:, 0:G].unsqueeze(2).broadcast_to((A, G, CAP3)),
                OP.is_equal)

            # ---- per-slot edge ids, broadcast, onehot sum ---------------
            for m in range(G):
                nc.tensor.matmul(gatt[:, m * SR:(m + 1) * SR],
                                 edeg_g[:, m * D:(m + 1) * D],
                                 pmm[:, m * SJ:m * SJ + SR])
            gatt_sb = pmid.tile([D, Q], bf16)
            nc.scalar.copy(gatt_sb[:], gatt)

            for j in range(4):
                nc.tensor.matmul(e04[:, j * Q:(j + 1) * Q],
                                 sel[:, j * A:(j + 1) * A], gatt_sb[:])
            nc.tensor.matmul(e4, sel[:, 4 * A:5 * A], gatt_sb[:])

            cmp = pmid.tile([A, 5 * Q], bf16)
            nc.vector.tensor_scalar(cmp[:, 0:4 * Q], e04[:], iotac, None,
                                    OP.is_equal)
            nc.vector.tensor_scalar(cmp[:, 4 * Q:5 * Q], e4, iotac,
                                    None, OP.is_equal)
            etc = pmid.tile([A, Q], bf16)
            with nc.allow_low_precision(reason="counts <= 6 exact in bf16"):
                nc.vector.tensor_reduce(
                    etc[:], cmp.rearrange("p (j q) -> p q j", j=5),
                    axis=AX.X, op=OP.add)
            et = pmid.tile([A, G * SJ], bf16)
            nc.gpsimd.memset(et[:], 0.0)
            ev = et.rearrange("p (m j) -> p m j", m=G)
            with nc.allow_low_precision(reason="counts <= 6 exact in bf16"):
                nc.vector.tensor_tensor(
                    ev[:, :, 0:SR], etc.rearrange("p (m s) -> p m s", m=G),
                    pv[:, :, 0:SR], OP.add)

            # ---- slot-major gathers: [atoms | sumbond] per molecule -----
            gts = []
            for t, (m0, cnt) in enumerate(TILES3):
                rows = SJ * cnt
                gt = ps_g.tile([96, FA + FB], f32, tag="gt", bufs=2)
                for k in range(cnt):
                    m = m0 + k
                    rb = SJ * k
                    nc.tensor.matmul(gt[rb:rb + SJ, 0:FA],
                                     et[:, m * SJ:(m + 1) * SJ],
                                     atoms_g[:, m * FA:(m + 1) * FA])
                    nc.tensor.matmul(gt[rb:rb + SJ, FA:FA + FB],
                                     pmm[:, m * SJ:(m + 1) * SJ],
                                     sb_g[:, m * FB:(m + 1) * FB])
                gt_sb = pmid.tile([96, FA + FB], bf16, tag="gts", bufs=3)
                nc.scalar.copy(gt_sb[0:rows, :], gt[0:rows, :])
                gts.append((gt_sb, rows))

            # ---- permute slot-major -> degree-block order ---------------
            for t in range(3):
                nc.tensor.matmul(f0p, gts[t][0][0:gts[t][1], 0:128],
                                 rts[t], start=(t == 0), stop=(t == 2))
            for t in range(3):
                nc.tensor.matmul(f1p, gts[t][0][0:gts[t][1], 128:256],
                                 rts[t], start=(t == 0), stop=(t == 2))
            for t in range(3):
                nc.tensor.matmul(f2p, gts[t][0][0:gts[t][1], 256:320],
                                 rts[t], start=(t == 0), stop=(t == 2))
            f0 = pmid.tile([128, Q], bf16)
            nc.scalar.copy(f0[:], f0p)
            f1 = pmid.tile([128, Q], bf16)
            nc.scalar.copy(f1[:], f1p)
            f2 = pmid.tile([FB + 1, Q], bf16)
            nc.scalar.copy(f2[0:FB, :], f2p)
            nc.gpsimd.memset(f2[FB:FB + 1, :], 1.0)

            # ---- dense: deg4 q[0,96) w cols [C,2C); deg3 [96,128) -------
            for zt, q0, q1, c0 in ((z4, 0, Q3_BASE, C),
                                   (z3, Q3_BASE, Q, 0)):
                nc.tensor.matmul(zt, f0[:, q0:q1], w0[:, c0:c0 + C],
                                 start=True, stop=False)
                nc.tensor.matmul(zt, f1[:, q0:q1], w1[:, c0:c0 + C],
                                 start=False, stop=False)
                nc.tensor.matmul(zt, f2[:, q0:q1], w2[:, c0:c0 + C],
                                 start=False, stop=True)

            out_sb = pout.tile([Q, C], bf16)
            nc.scalar.activation(out_sb[0:Q3_BASE, :], z4, AF.Relu)
            nc.scalar.activation(out_sb[Q3_BASE:Q, :], z3, AF.Relu)
            nc.gpsimd.dma_start(out=out_ap[bg], in_=out_sb[:])

    nc.compile()
    return nc


def _get_nc():
    if "nc" not in _CACHE:
        _CACHE["nc"] = _build_program()
    return _CACHE["nc"]


def _make_in_maps(atoms, bonds, edges, W, b):
    import ml_dtypes

    bf16 = ml_dtypes.bfloat16
    atoms = np.asarray(atoms, dtype=np.float32)
    bonds = np.asarray(bonds, dtype=np.float32)
    edges = np.asarray(edges)
    W = np.asarray(W, dtype=np.float32)
    b = np.asarray(b, dtype=np.float32)

    # group-major layouts: (core, group, A, G*feat)
    def grp(x, feat):
        return np.ascontiguousarray(
            x.reshape(NCORES, NG, G, A, feat).transpose(0, 1, 3, 2, 4)
            .reshape(NCORES, NG, A, G * feat).astype(bf16))

    atoms_h = grp(atoms, FA)
    bonds_h = grp(bonds.reshape(B, A, D * FB), D * FB)
    edeg_h = grp(edges.astype(np.float32), D)

    # weights for degrees (3, 4): cols [0,C) = deg3, [C,2C) = deg4
    waug = np.concatenate([W, b[:, None, :]], axis=1)       # (5, 321, 256)
    w34 = waug[[3, 4]]                                       # (2, 321, 256)
    wpack = np.zeros((128, 6 * C), dtype=np.float32)
    wpack[:, 0:2 * C] = w34[:, 0:128].transpose(1, 0, 2).reshape(128, 2 * C)
    wpack[:, 2 * C:4 * C] = w34[:, 128:256].transpose(1, 0, 2).reshape(
        128, 2 * C)
    wpack[0:FB + 1, 4 * C:6 * C] = w34[:, 256:321].transpose(1, 0, 2).reshape(
        FB + 1, 2 * C)
    wpack = wpack.astype(bf16)

    cpack = np.zeros((A, C_TOT), dtype=np.float32)
    # ltri[k, m] = 1 if k <= m  (inclusive prefix sums via ltri^T @ mask)
    cpack[:, C_LTRI:C_LTRI + A] = np.triu(np.ones((A, A), dtype=np.float32))
    iotaj_row = np.zeros(G * SJ, dtype=np.float32)
    for m in range(G):
        for j in range(SR):
            iotaj_row[m * SJ + j] = (j + 1) if j < CAP4 else (j - CAP4 + 1)
    cpack[:, C_IOTAJ:C_IOTAJ + G * SJ] = iotaj_row
    cpack[:, C_LADDER:C_LADDER + 2 * G] = np.array(
        [3.0] * G + [4.0] * G, dtype=np.float32)
    for j in range(D):
        cpack[j, C_SEL + j * A:C_SEL + (j + 1) * A] = 1.0
    cpack = cpack.astype(bf16)

    rpack = np.zeros((96, 3 * Q), dtype=np.float32)
    for t, (m0, cnt) in enumerate(TILES3):
        for k in range(cnt):
            m = m0 + k
            for j in range(SR):
                q = (m * CAP4 + j) if j < CAP4 else (
                    Q3_BASE + m * CAP3 + (j - CAP4))
                rpack[SJ * k + j, t * Q + q] = 1.0
    rpack = rpack.astype(bf16)

    return [
        {
            "atoms": atoms_h[c],
            "bonds": bonds_h[c],
            "edeg": edeg_h[c],
            "cpack": cpack, "rpack": rpack, "wpack": wpack,
            "iotac": np.arange(A, dtype=np.float32).reshape(A, 1),
        }
        for c in range(NCORES)
    ]


def _assemble(out_hw, atoms, bonds, edges, W, b):
    """Scatter compact HW rows into the full output; numpy fallback for
    atoms outside the static slot capacities (deg<=2 or rank overflow)."""
    atoms = np.asarray(atoms, dtype=np.float32)
    bonds = np.asarray(bonds, dtype=np.float32)
    edges = np.asarray(edges)
    W = np.asarray(W, dtype=np.float32)
    b = np.asarray(b, dtype=np.float32)

    deg = (edges != -1).sum(-1)                         # (B, A)
    out = np.zeros((B, A, C), dtype=np.float32)
    covered = np.zeros((B, A), dtype=bool)
    gi = np.arange(B) // G                              # global group index
    ii = np.arange(B) % G                               # molecule in group

    for d, cap, base in ((4, CAP4, 0), (3, CAP3, Q3_BASE)):
        mask = deg == d
        rank = np.cumsum(mask, axis=1)
        ok = mask & (rank <= cap)
        mi, ai = np.nonzero(ok)
        q = base + ii[mi] * cap + (rank[mi, ai] - 1)
        out[mi, ai] = out_hw[gi[mi], q].astype(np.float32)
        covered |= ok

    rest = (deg < D) & ~covered
    for m, a in zip(*np.nonzero(rest)):
        e = edges[m, a]
        e = e[e >= 0]
        fa = atoms[m, a] + (atoms[m, e].sum(0) if e.size else 0.0)
        feat = np.concatenate([fa, bonds[m, a].sum(0)])
        z = feat @ W[deg[m, a]] + b[deg[m, a]]
        out[m, a] = np.maximum(z, 0.0)
    return out


def run_sharded(atoms, bonds, edges, W, b, trace=False):
    """Run on the 8 NeuronCores; returns (output, BassKernelResults)."""
    from concourse.bass_utils import run_bass_kernel_spmd

    nc = _get_nc()
    in_maps = _make_in_maps(atoms, bonds, edges, W, b)
    res = run_bass_kernel_spmd(nc, in_maps, list(range(NCORES)), trace=trace)
    out_hw = np.concatenate(
        [np.asarray(res.results[c]["out"]) for c in range(NCORES)],
        axis=0)                                          # (NCORES*NG, Q, C)
    out = _assemble(out_hw, atoms, bonds, edges, W, b)
    return out, res


def kernel(atoms, bonds, edges, W, b):
    out, _ = run_sharded(atoms, bonds, edges, W, b)
    return out


# revision 29
# speedup vs baseline: 1.5477x; 1.5477x over previous
"""Trainium2 Bass kernel for NeuralGraphHidden (GNN message passing).

Full-input contract: kernel(**inputs) takes the complete unsharded arrays,
shards batch dim 0 across 8 NeuronCores (data parallel), runs one SPMD Bass
program, and reassembles the full output.

Key structural fact exploited: deg[a] = #(edges[a,:] != -1) is in 0..5, but
the reference's degree mask covers only 0..4 - atoms with deg==5 (about 96%
of atoms for this input distribution) produce an all-zero output row.  The
kernel compacts the few deg<5 atoms per molecule into static per-degree
slots on-chip and runs the dense layer only on those slots:

  per group of 8 molecules (4 groups/core, 32 molecules/core):
    sumbond   = bonds pre-summed over the 5 bond slots by the
                DMA compute engine (accum_op=add)             (DMA CCE)
    deg       = row counts of edges != -1                     (DVE)
    rank_d    = per-degree prefix sums via tri-matmul          (PE)
    P         = slot one-hot (atom -> slot), 32-padded         (DVE)
    gatt      = per-slot edge ids via edeg^T @ P               (PE)
    E_j       = edge ids broadcast down partitions             (PE)
    ET        = sum_j onehot(E_j) + P  (neighbour+self)        (DVE)
    nstt|braw = [atoms | sumbond] gathered per slot            (PE)
    f0/f1/f2  = feats re-permuted slot->degree-block via
                constant permutation matmuls                   (PE)
    z4/z3     = per-degree dense with W_4 / W_3 (+bias row)    (PE)
    out       = relu(z)                                        (ScalarE)

  Matmul PSUM outputs may only start at partition 0/32/64, so per-molecule
  slot rows are 32-padded and grouped 3+3+2 molecules into three gather
  tiles; the dense stage uses two tiles (deg4: 96 rows, deg3: 32 rows).
  Slot capacities: 12 deg-4 + 4 deg-3 per molecule (measured maxima 12/2).
  deg<=2 atoms and any capacity overflow fall back to a tiny numpy path on
  the host (0-1 atoms in practice).  The host scatters the compact HW rows
  into the zero-initialised full output.
"""

import sys

sys.path.insert(0, "/opt/trn_rl_repo")

import numpy as np

B, A, D = 256, 128, 5
FA, FB, C = 256, 64, 256
NCORES = 8
BL = B // NCORES   # 32 molecules per core
G = 8              # molecules per group
NG = BL // G       # 4 groups per core

SJ = 32            # padded slots per molecule (16 real: 12 deg4 + 4 deg3)
SR = 16            # real slots per molecule
CAP4, CAP3 = 12, 4
Q = G * SR         # 128 dense-stage slots per group
Q3_BASE = G * CAP4  # 96: deg3 block starts here in q space
TILES3 = ((0, 3), (3, 3), (6, 2))   # (first molecule, count) per gather tile

# packed-const column offsets (cpack, bf16, 128 partitions)
C_LTRI = 0          # (128, 128)
C_IOTAJ = 128       # (128, 256)
C_LADDER = 384      # (128, 16)
C_IOTAC = 400       # (128, 1)
C_SEL = 401         # (5, 640)
C_TOT = 1041

_CACHE = {}


def _build_program():
    from contextlib import ExitStack

    import concourse.bass as bass
    import concourse.tile as tile
    from concourse import bacc, mybir

    f32 = mybir.dt.float32
    bf16 = mybir.dt.bfloat16
    AF = mybir.ActivationFunctionType
    OP = mybir.AluOpType
    AX = mybir.AxisListType

    nc = bacc.Bacc("TRN2", target_bir_lowering=False, debug=False,
                   num_devices=NCORES)

    atoms_d = nc.dram_tensor("atoms", [NG, A, G * FA], bf16,
                             kind="ExternalInput")
    bonds_d = nc.dram_tensor("bonds", [NG, A, G * D * FB], bf16,
                             kind="ExternalInput")
    edeg_d = nc.dram_tensor("edeg", [NG, A, G * D], bf16,
                            kind="ExternalInput")
    cpack_d = nc.dram_tensor("cpack", [A, C_TOT], bf16, kind="ExternalInput")
    iotac_d = nc.dram_tensor("iotac", [A, 1], f32, kind="ExternalInput")
    rpack_d = nc.dram_tensor("rpack", [96, 3 * Q], bf16,
                             kind="ExternalInput")
    wpack_d = nc.dram_tensor("wpack", [128, 3 * 2 * C], bf16,
                             kind="ExternalInput")
    out_d = nc.dram_tensor("out", [NG, Q, C], bf16, kind="ExternalOutput")

    atoms_ap = atoms_d.ap()
    bonds_ap = bonds_d.ap()
    edeg_ap = edeg_d.ap()
    out_ap = out_d.ap()

    with tile.TileContext(nc) as tc, ExitStack() as ctx:
        consts = ctx.enter_context(tc.tile_pool(name="consts", bufs=1))
        pin = ctx.enter_context(tc.tile_pool(name="pin", bufs=2))
        pmid = ctx.enter_context(tc.tile_pool(name="pmid", bufs=2))
        pout = ctx.enter_context(tc.tile_pool(name="pout", bufs=2))
        ps_e = ctx.enter_context(
            tc.tile_pool(name="ps_e", bufs=1, space="PSUM"))
        ps_g = ctx.enter_context(
            tc.tile_pool(name="ps_g", bufs=1, space="PSUM"))

        # ---- packed constants: 3 DMAs total -----------------------------
        cpack = consts.tile([A, C_TOT], bf16)
        nc.scalar.dma_start(out=cpack[:], in_=cpack_d.ap()[:])
        rpack = consts.tile([96, 3 * Q], bf16)
        nc.scalar.dma_start(out=rpack[:], in_=rpack_d.ap()[:])
        wpack = consts.tile([128, 3 * 2 * C], bf16)
        nc.scalar.dma_start(out=wpack[:], in_=wpack_d.ap()[:])
        iotac_t = consts.tile([A, 1], f32)
        nc.scalar.dma_start(out=iotac_t[:], in_=iotac_d.ap()[:])

        ltri = cpack[:, C_LTRI:C_LTRI + 128]
        iotaj = cpack[:, C_IOTAJ:C_IOTAJ + G * SJ]
        ladder = cpack[:, C_LADDER:C_LADDER + 2 * G]
        iotac = iotac_t[:]
        sel = cpack[0:5, C_SEL:C_SEL + 5 * A]
        rts = [rpack[0:96, 0:Q], rpack[0:96, Q:2 * Q],
               rpack[0:64, 2 * Q:3 * Q]]
        w0 = wpack[:, 0:2 * C]
        w1 = wpack[:, 2 * C:4 * C]
        w2 = wpack[0:FB + 1, 4 * C:6 * C]

        for bg in range(NG):
            # ---- inputs; bonds pre-summed over d by DMA CCE -------------
            edeg_g = pin.tile([A, G * D], bf16)
            nc.sync.dma_start(out=edeg_g[:], in_=edeg_ap[bg])
            atoms_g = pin.tile([A, G * FA], bf16)
            nc.sync.dma_start(out=atoms_g[:], in_=atoms_ap[bg])
            bonds_g = pin.tile([A, G * D * FB], bf16)
            nc.sync.dma_start(out=bonds_g[:], in_=bonds_ap[bg])

            # PSUM bank packing (8 banks of 2KB):
            #   small1 x2, e04 x2, misc, z3t, gt x2
            small1 = ps_e.tile([A, 400], f32, tag="small1", bufs=1)
            pos2 = small1[:, 0:16]
            gatt = small1[0:5, 16:144]
            f0p = small1[:, 144:272]
            e4 = small1[:, 272:400]
            e04 = ps_e.tile([A, 4 * Q], f32, tag="e04", bufs=1)
            misc = ps_e.tile([A, 384], f32, tag="misc", bufs=1)
            f2p = misc[0:FB, 0:Q]
            z4 = misc[0:Q3_BASE, Q:Q + C]
            z3t = ps_e.tile([A, 384], f32, tag="z3t", bufs=1)
            z3 = z3t[0:Q - Q3_BASE, 0:C]
            f1p = z3t[:, C:C + Q]

            # ---- degree + per-degree ranks ------------------------------
            ne = pmid.tile([A, G * D], bf16)
            nc.vector.tensor_scalar(ne[:], edeg_g[:], -1.0, None,
                                    OP.not_equal)
            deg = pmid.tile([A, G], bf16)
            with nc.allow_low_precision(reason="counts <= 5 exact in bf16"):
                nc.vector.tensor_reduce(
                    deg[:], ne.rearrange("p (g d) -> p g d", g=G),
                    axis=AX.X, op=OP.add)
            masks2 = pmid.tile([A, 2 * G], bf16)
            nc.vector.tensor_tensor(
                masks2.rearrange("p (d g) -> p d g", d=2),
                ladder.rearrange("p (d g) -> p d g", d=2),
                deg.unsqueeze(1).broadcast_to((A, 2, G)),
                OP.is_equal)
            nc.tensor.matmul(pos2, ltri, masks2[:])
            posm2 = pmid.tile([A, 2 * G], bf16)
            with nc.allow_low_precision(reason="ranks <= 128 exact in bf16"):
                nc.vector.tensor_tensor(posm2[:], pos2, masks2[:], OP.mult)

            # ---- slot one-hot P, 32-padded per molecule -----------------
            pmm = pmid.tile([A, G * SJ], bf16)
            nc.gpsimd.memset(pmm[:], 0.0)
            pv = pmm.rearrange("p (m j) -> p m j", m=G)
            iv = iotaj.rearrange("p (m j) -> p m j", m=G)
            nc.vector.tensor_tensor(
                pv[:, :, 0:CAP4], iv[:, :, 0:CAP4],
                posm2[:, G:2 * G].unsqueeze(2).broadcast_to((A, G, CAP4)),
                OP.is_equal)
            nc.vector.tensor_tensor(
                pv[:, :, CAP4:SR], iv[:, :, CAP4:SR],
                posm2[:, 0:G].unsqueeze(2).broadcast_to((A, G, CAP3)),
                OP.is_equal)

            # ---- per-slot edge ids, broadcast, onehot sum ---------------
            for m in range(G):
                nc.tensor.matmul(gatt[:, m * SR:(m + 1) * SR],
                                 edeg_g[:, m * D:(m + 1) * D],
                                 pmm[:, m * SJ:m * SJ + SR])
            gatt_sb = pmid.tile([D, Q], bf16)
            nc.scalar.copy(gatt_sb[:], gatt)

            for j in range(4):
                nc.tensor.matmul(e04[:, j * Q:(j + 1) * Q],
                                 sel[:, j * A:(j + 1) * A], gatt_sb[:])
            nc.tensor.matmul(e4, sel[:, 4 * A:5 * A], gatt_sb[:])

            cmp = pmid.tile([A, 5 * Q], bf16)
            nc.vector.tensor_scalar(cmp[:, 0:4 * Q], e04[:], iotac, None,
                                    OP.is_equal)
            nc.vector.tensor_scalar(cmp[:, 4 * Q:5 * Q], e4, iotac,
                                    None, OP.is_equal)
            etc = pmid.tile([A, Q], bf16)
            with nc.allow_low_precision(reason="counts <= 6 exact in bf16"):
                nc.vector.tensor_reduce(
                    etc[:], cmp.rearrange("p (j q) -> p q j", j=5),
                    axis=AX.X, op=OP.add)
            et = pmid.tile([A, G * SJ], bf16)
            nc.gpsimd.memset(et[:], 0.0)
            ev = et.rearrange("p (m j) -> p m j", m=G)
            with nc.allow_low_precision(reason="counts <= 6 exact in bf16"):
                nc.vector.tensor_tensor(
                    ev[:, :, 0:SR], etc.rearrange("p (m s) -> p m s", m=G),
                    pv[:, :, 0:SR], OP.add)

            # ---- slot-major gathers (3 tiles, 32-row padded molecules) --
            nstts, bsums = [], []
            for t, (m0, cnt) in enumerate(TILES3):
                rows = SJ * cnt
                nstt = ps_g.tile([96, FA], f32, tag="nstt", bufs=2)
                braw = ps_g.tile([96, D * FB], f32, tag="braw", bufs=2)
                for k in range(cnt):
                    m = m0 + k
                    rb = SJ * k
                    nc.tensor.matmul(nstt[rb:rb + SJ, :],
                                     et[:, m * SJ:(m + 1) * SJ],
                                     atoms_g[:, m * FA:(m + 1) * FA])
                    nc.tensor.matmul(
                        braw[rb:rb + SJ, :],
                        pmm[:, m * SJ:(m + 1) * SJ],
                        bonds_g[:, m * D * FB:(m + 1) * D * FB])
                nstt_sb = pmid.tile([96, FA], bf16, tag="nstts", bufs=3)
                nc.scalar.copy(nstt_sb[0:rows, :], nstt[0:rows, :])
                bsum = pmid.tile([96, FB], bf16, tag="bsum", bufs=3)
                with nc.allow_low_precision(reason="bf16 bond sums"):
                    nc.vector.tensor_reduce(
                        bsum[0:rows, :],
                        braw[0:rows, :].rearrange("p (d f) -> p f d", d=D),
                        axis=AX.X, op=OP.add)
                nstts.append((nstt_sb, rows))
                bsums.append(bsum)

            # ---- permute slot-major -> degree-block order ---------------
            for t in range(3):
                nc.tensor.matmul(f0p, nstts[t][0][0:nstts[t][1], 0:128],
                                 rts[t], start=(t == 0), stop=(t == 2))
            for t in range(3):
                nc.tensor.matmul(f1p, nstts[t][0][0:nstts[t][1], 128:256],
                                 rts[t], start=(t == 0), stop=(t == 2))
            for t in range(3):
                nc.tensor.matmul(f2p, bsums[t][0:nstts[t][1], :],
                                 rts[t], start=(t == 0), stop=(t == 2))
            f0 = pmid.tile([128, Q], bf16)
            nc.scalar.copy(f0[:], f0p)
            f1 = pmid.tile([128, Q], bf16)
            nc.scalar.copy(f1[:], f1p)
            f2 = pmid.tile([FB + 1, Q], bf16)
            nc.scalar.copy(f2[0:FB, :], f2p)
            nc.gpsimd.memset(f2[FB:FB + 1, :], 1.0)

            # ---- dense: deg4 q[0,96) w cols [C,2C); deg3 [96,128) -------
            for zt, q0, q1, c0 in ((z4, 0, Q3_BASE, C),
                                   (z3, Q3_BASE, Q, 0)):
                nc.tensor.matmul(zt, f0[:, q0:q1], w0[:, c0:c0 + C],
                                 start=True, stop=False)
                nc.tensor.matmul(zt, f1[:, q0:q1], w1[:, c0:c0 + C],
                                 start=False, stop=False)
                nc.tensor.matmul(zt, f2[:, q0:q1], w2[:, c0:c0 + C],
                                 start=False, stop=True)

            out_sb = pout.tile([Q, C], bf16)
            nc.scalar.activation(out_sb[0:Q3_BASE, :], z4, AF.Relu)
            nc.scalar.activation(out_sb[Q3_BASE:Q, :], z3, AF.Relu)
            nc.gpsimd.dma_start(out=out_ap[bg], in_=out_sb[:])

    nc.compile()
    return nc


def _get_nc():
    if "nc" not in _CACHE:
        _CACHE["nc"] = _build_program()
    return _CACHE["nc"]


def _make_in_maps(atoms, bonds, edges, W, b):
    import ml_dtypes

    bf16 = ml_dtypes.bfloat16
    atoms = np.asarray(atoms, dtype=np.float32)
    bonds = np.asarray(bonds, dtype=np.float32)
    edges = np.asarray(edges)
    W = np.asarray(W, dtype=np.float32)
    b = np.asarray(b, dtype=np.float32)

    # group-major layouts: (core, group, A, G*feat)
    def grp(x, feat):
        return np.ascontiguousarray(
            x.reshape(NCORES, NG, G, A, feat).transpose(0, 1, 3, 2, 4)
            .reshape(NCORES, NG, A, G * feat).astype(bf16))

    atoms_h = grp(atoms, FA)
    bonds_h = grp(bonds.reshape(B, A, D * FB), D * FB)
    edeg_h = grp(edges.astype(np.float32), D)

    # weights for degrees (3, 4): cols [0,C) = deg3, [C,2C) = deg4
    waug = np.concatenate([W, b[:, None, :]], axis=1)       # (5, 321, 256)
    w34 = waug[[3, 4]]                                       # (2, 321, 256)
    wpack = np.zeros((128, 6 * C), dtype=np.float32)
    wpack[:, 0:2 * C] = w34[:, 0:128].transpose(1, 0, 2).reshape(128, 2 * C)
    wpack[:, 2 * C:4 * C] = w34[:, 128:256].transpose(1, 0, 2).reshape(
        128, 2 * C)
    wpack[0:FB + 1, 4 * C:6 * C] = w34[:, 256:321].transpose(1, 0, 2).reshape(
        FB + 1, 2 * C)
    wpack = wpack.astype(bf16)

    cpack = np.zeros((A, C_TOT), dtype=np.float32)
    # ltri[k, m] = 1 if k <= m  (inclusive prefix sums via ltri^T @ mask)
    cpack[:, C_LTRI:C_LTRI + A] = np.triu(np.ones((A, A), dtype=np.float32))
    iotaj_row = np.zeros(G * SJ, dtype=np.float32)
    for m in range(G):
        for j in range(SR):
            iotaj_row[m * SJ + j] = (j + 1) if j < CAP4 else (j - CAP4 + 1)
    cpack[:, C_IOTAJ:C_IOTAJ + G * SJ] = iotaj_row
    cpack[:, C_LADDER:C_LADDER + 2 * G] = np.array(
        [3.0] * G + [4.0] * G, dtype=np.float32)
    for j in range(D):
        cpack[j, C_SEL + j * A:C_SEL + (j + 1) * A] = 1.0
    cpack = cpack.astype(bf16)

    rpack = np.zeros((96, 3 * Q), dtype=np.float32)
    for t, (m0, cnt) in enumerate(TILES3):
        for k in range(cnt):
            m = m0 + k
            for j in range(SR):
                q = (m * CAP4 + j) if j < CAP4 else (
                    Q3_BASE + m * CAP3 + (j - CAP4))
                rpack[SJ * k + j, t * Q + q] = 1.0
    rpack = rpack.astype(bf16)

    return [
        {
            "atoms": atoms_h[c],
            "bonds": bonds_h[c],
            "edeg": edeg_h[c],
            "cpack": cpack, "rpack": rpack, "wpack": wpack,
            "iotac": np.arange(A, dtype=np.float32).reshape(A, 1),
        }
        for c in range(NCORES)
    ]


def _assemble(out_hw, atoms, bonds, edges, W, b):
    """Scatter compact HW rows into the full output; numpy fallback for
    atoms outside the static slot capacities (deg<=2 or rank overflow)."""
    atoms = np.asarray(atoms, dtype=np.float32)
    bonds = np.asarray(bonds, dtype=np.float32)
    edges = np.asarray(edges)
    W = np.asarray(W, dtype=np.float32)
    b = np.asarray(b, dtype=np.float32)

    deg = (edges != -1).sum(-1)                         # (B, A)
    out = np.zeros((B, A, C), dtype=np.float32)
    covered = np.zeros((B, A), dtype=bool)
    gi = np.arange(B) // G                              # global group index
    ii = np.arange(B) % G                               # molecule in group

    for d, cap, base in ((4, CAP4, 0), (3, CAP3, Q3_BASE)):
        mask = deg == d
        rank = np.cumsum(mask, axis=1)
        ok = mask & (rank <= cap)
        mi, ai = np.nonzero(ok)
        q = base + ii[mi] * cap + (rank[mi, ai] - 1)
        out[mi, ai] = out_hw[gi[mi], q].astype(np.float32)
        covered |= ok

    rest = (deg < D) & ~covered
    for m, a in zip(*np.nonzero(rest)):
        e = edges[m, a]
        e = e[e >= 0]
        fa = atoms[m, a] + (atoms[m, e].sum(0) if e.size else 0.0)
        feat = np.concatenate([fa, bonds[m, a].sum(0)])
        z = feat @ W[deg[m, a]] + b[deg[m, a]]
        out[m, a] = np.maximum(z, 0.0)
    return out


def run_sharded(atoms, bonds, edges, W, b, trace=False):
    """Run on the 8 NeuronCores; returns (output, BassKernelResults)."""
    from concourse.bass_utils import run_bass_kernel_spmd

    nc = _get_nc()
    in_maps = _make_in_maps(atoms, bonds, edges, W, b)
    res = run_bass_kernel_spmd(nc, in_maps, list(range(NCORES)), trace=trace)
    out_hw = np.concatenate(
        [np.asarray(res.results[c]["out"]) for c in range(NCORES)],
        axis=0)                                          # (NCORES*NG, Q, C)
    out = _assemble(out_hw, atoms, bonds, edges, W, b)
    return out, res


def kernel(atoms, bonds, edges, W, b):
    out, _ = run_sharded(atoms, bonds, edges, W, b)
    return out


# revision 32
# speedup vs baseline: 1.5600x; 1.0079x over previous
"""Trainium2 Bass kernel for NeuralGraphHidden (GNN message passing).

Full-input contract: kernel(**inputs) takes the complete unsharded arrays,
shards batch dim 0 across 8 NeuronCores (data parallel), runs one SPMD Bass
program, and reassembles the full output.

Key structural fact exploited: deg[a] = #(edges[a,:] != -1) is in 0..5, but
the reference's degree mask covers only 0..4 - atoms with deg==5 (about 96%
of atoms for this input distribution) produce an all-zero output row.  The
kernel compacts the few deg<5 atoms per molecule into static per-degree
slots on-chip and runs the dense layer only on those slots:

  per group of 8 molecules (4 groups/core, 32 molecules/core):
    sumbond   = bonds pre-summed over the 5 bond slots by the
                DMA compute engine (accum_op=add)             (DMA CCE)
    deg       = row counts of edges != -1                     (DVE)
    rank_d    = per-degree prefix sums via tri-matmul          (PE)
    P         = slot one-hot (atom -> slot), 32-padded         (DVE)
    gatt      = per-slot edge ids via edeg^T @ P               (PE)
    E_j       = edge ids broadcast down partitions             (PE)
    ET        = sum_j onehot(E_j) + P  (neighbour+self)        (DVE)
    nstt|braw = [atoms | sumbond] gathered per slot            (PE)
    f0/f1/f2  = feats re-permuted slot->degree-block via
                constant permutation matmuls                   (PE)
    z4/z3     = per-degree dense with W_4 / W_3 (+bias row)    (PE)
    out       = relu(z)                                        (ScalarE)

  Matmul PSUM outputs may only start at partition 0/32/64, so per-molecule
  slot rows are 32-padded and grouped 3+3+2 molecules into three gather
  tiles; the dense stage uses two tiles (deg4: 96 rows, deg3: 32 rows).
  Slot capacities: 12 deg-4 + 4 deg-3 per molecule (measured maxima 12/2).
  deg<=2 atoms and any capacity overflow fall back to a tiny numpy path on
  the host (0-1 atoms in practice).  The host scatters the compact HW rows
  into the zero-initialised full output.
"""

import sys

sys.path.insert(0, "/opt/trn_rl_repo")

import numpy as np

B, A, D = 256, 128, 5
FA, FB, C = 256, 64, 256
NCORES = 8
BL = B // NCORES   # 32 molecules per core
G = 8              # molecules per group
NG = BL // G       # 4 groups per core

SJ = 32            # padded slots per molecule (16 real: 12 deg4 + 4 deg3)
SR = 16            # real slots per molecule
CAP4, CAP3 = 12, 4
Q = G * SR         # 128 dense-stage slots per group
Q3_BASE = G * CAP4  # 96: deg3 block starts here in q space
TILES3 = ((0, 3), (3, 3), (6, 2))   # (first molecule, count) per gather tile

# packed-const column offsets (cpack, bf16, 128 partitions)
C_LTRI = 0          # (128, 128)
C_IOTAJ = 128       # (128, 256)
C_LADDER = 384      # (128, 16)
C_SEL = 400         # (6, 768)
C_TOT = 1168

_CACHE = {}


def _build_program():
    from contextlib import ExitStack

    import concourse.bass as bass
    import concourse.tile as tile
    from concourse import bacc, mybir

    f32 = mybir.dt.float32
    bf16 = mybir.dt.bfloat16
    AF = mybir.ActivationFunctionType
    OP = mybir.AluOpType
    AX = mybir.AxisListType

    nc = bacc.Bacc("TRN2", target_bir_lowering=False, debug=False,
                   num_devices=NCORES)

    atoms_d = nc.dram_tensor("atoms", [NG, A, G * FA], bf16,
                             kind="ExternalInput")
    bonds_d = nc.dram_tensor("bonds", [NG, A, G * D * FB], bf16,
                             kind="ExternalInput")
    edeg_d = nc.dram_tensor("edeg", [NG, A, G * 6], bf16,
                            kind="ExternalInput")
    cpack_d = nc.dram_tensor("cpack", [A, C_TOT], bf16, kind="ExternalInput")
    iotac_d = nc.dram_tensor("iotac", [A, 1], f32, kind="ExternalInput")
    rpack_d = nc.dram_tensor("rpack", [96, 3 * Q], bf16,
                             kind="ExternalInput")
    wpack_d = nc.dram_tensor("wpack", [128, 3 * 2 * C], bf16,
                             kind="ExternalInput")
    out_d = nc.dram_tensor("out", [NG, Q, C], bf16, kind="ExternalOutput")

    atoms_ap = atoms_d.ap()
    bonds_ap = bonds_d.ap()
    edeg_ap = edeg_d.ap()
    out_ap = out_d.ap()

    with tile.TileContext(nc) as tc, ExitStack() as ctx:
        consts = ctx.enter_context(tc.tile_pool(name="consts", bufs=1))
        pin = ctx.enter_context(tc.tile_pool(name="pin", bufs=2))
        pmid = ctx.enter_context(tc.tile_pool(name="pmid", bufs=2))
        pout = ctx.enter_context(tc.tile_pool(name="pout", bufs=2))
        ps_e = ctx.enter_context(
            tc.tile_pool(name="ps_e", bufs=1, space="PSUM"))
        ps_g = ctx.enter_context(
            tc.tile_pool(name="ps_g", bufs=1, space="PSUM"))

        # ---- packed constants: 3 DMAs total -----------------------------
        cpack = consts.tile([A, C_TOT], bf16)
        nc.scalar.dma_start(out=cpack[:], in_=cpack_d.ap()[:])
        rpack = consts.tile([96, 3 * Q], bf16)
        nc.scalar.dma_start(out=rpack[:], in_=rpack_d.ap()[:])
        wpack = consts.tile([128, 3 * 2 * C], bf16)
        nc.scalar.dma_start(out=wpack[:], in_=wpack_d.ap()[:])
        iotac_t = consts.tile([A, 1], f32)
        nc.scalar.dma_start(out=iotac_t[:], in_=iotac_d.ap()[:])

        ltri = cpack[:, C_LTRI:C_LTRI + 128]
        iotaj = cpack[:, C_IOTAJ:C_IOTAJ + G * SJ]
        ladder = cpack[:, C_LADDER:C_LADDER + 2 * G]
        iotac = iotac_t[:]
        sel = cpack[0:6, C_SEL:C_SEL + 6 * A]
        rts = [rpack[0:96, 0:Q], rpack[0:96, Q:2 * Q],
               rpack[0:64, 2 * Q:3 * Q]]
        w0 = wpack[:, 0:2 * C]
        w1 = wpack[:, 2 * C:4 * C]
        w2 = wpack[0:FB + 1, 4 * C:6 * C]

        for bg in range(NG):
            # ---- inputs; bonds pre-summed over d by DMA CCE -------------
            edeg_g = pin.tile([A, G * 6], bf16)
            nc.sync.dma_start(out=edeg_g[:], in_=edeg_ap[bg])
            atoms_g = pin.tile([A, G * FA], bf16)
            nc.sync.dma_start(out=atoms_g[:], in_=atoms_ap[bg])
            bonds_g = pin.tile([A, G * D * FB], bf16)
            nc.sync.dma_start(out=bonds_g[:], in_=bonds_ap[bg])

            # PSUM bank packing (8 banks of 2KB):
            #   early x2, e04 x1, zf x2, nstt x2, braw x1
            early = ps_e.tile([A, 400], f32, tag="early", bufs=2)
            pos2 = early[:, 0:16]
            gatt = early[0:6, 16:144]
            e45 = early[:, 144:400]
            e04 = ps_e.tile([A, 4 * Q], f32, tag="e04", bufs=1)
            zf = ps_e.tile([A, 512], f32, tag="zf", bufs=2)
            f0p = zf[:, 0:Q]
            f1p = zf[:, Q:2 * Q]
            f2p = zf[0:FB, 2 * Q:3 * Q]
            z4 = zf[0:Q3_BASE, 0:C]
            z3 = zf[0:Q - Q3_BASE, C:2 * C]

            # ---- degree + per-degree ranks ------------------------------
            ne = pmid.tile([A, G * D], bf16)
            nc.vector.tensor_scalar(
                ne.rearrange("p (g d) -> p g d", g=G),
                edeg_g.rearrange("p (g d) -> p g d", g=G)[:, :, 0:D],
                -1.0, None, OP.not_equal)
            deg = pmid.tile([A, G], bf16)
            with nc.allow_low_precision(reason="counts <= 5 exact in bf16"):
                nc.vector.tensor_reduce(
                    deg[:], ne.rearrange("p (g d) -> p g d", g=G),
                    axis=AX.X, op=OP.add)
            masks2 = pmid.tile([A, 2 * G], bf16)
            nc.vector.tensor_tensor(
                masks2.rearrange("p (d g) -> p d g", d=2),
                ladder.rearrange("p (d g) -> p d g", d=2),
                deg.unsqueeze(1).broadcast_to((A, 2, G)),
                OP.is_equal)
            nc.tensor.matmul(pos2, ltri, masks2[:])
            posm2 = pmid.tile([A, 2 * G], bf16)
            with nc.allow_low_precision(reason="ranks <= 128 exact in bf16"):
                nc.vector.tensor_tensor(posm2[:], pos2, masks2[:], OP.mult)

            # ---- slot one-hot P, 32-padded per molecule -----------------
            pmm = pmid.tile([A, G * SJ], bf16)
            nc.gpsimd.memset(pmm[:], 0.0)
            pv = pmm.rearrange("p (m j) -> p m j", m=G)
            iv = iotaj.rearrange("p (m j) -> p m j", m=G)
            nc.vector.tensor_tensor(
                pv[:, :, 0:CAP4], iv[:, :, 0:CAP4],
                posm2[:, G:2 * G].unsqueeze(2).broadcast_to((A, G, CAP4)),
                OP.is_equal)
            nc.vector.tensor_tensor(
                pv[:, :, CAP4:SR], iv[:, :, CAP4:SR],
                posm2[:, 0:G].unsqueeze(2).broadcast_to((A, G, CAP3)),
                OP.is_equal)

            # ---- per-slot edge ids, broadcast, onehot sum ---------------
            for m in range(G):
                nc.tensor.matmul(gatt[:, m * SR:(m + 1) * SR],
                                 edeg_g[:, m * 6:(m + 1) * 6],
                                 pmm[:, m * SJ:m * SJ + SR])
            gatt_sb = pmid.tile([6, Q], bf16)
            nc.scalar.copy(gatt_sb[:], gatt)

            for j in range(4):
                nc.tensor.matmul(e04[:, j * Q:(j + 1) * Q],
                                 sel[:, j * A:(j + 1) * A], gatt_sb[:])
            for j in (4, 5):
                nc.tensor.matmul(e45[:, (j - 4) * Q:(j - 3) * Q],
                                 sel[:, j * A:(j + 1) * A], gatt_sb[:])

            cmp = pmid.tile([A, 6 * Q], bf16)
            nc.vector.tensor_scalar(cmp[:, 0:4 * Q], e04[:], iotac, None,
                                    OP.is_equal)
            nc.vector.tensor_scalar(cmp[:, 4 * Q:6 * Q], e45, iotac,
                                    None, OP.is_equal)
            et = pmid.tile([A, G * SJ], bf16)
            nc.gpsimd.memset(et[:], 0.0)
            ev = et.rearrange("p (m j) -> p m j", m=G)
            with nc.allow_low_precision(reason="counts <= 6 exact in bf16"):
                nc.vector.tensor_reduce(
                    ev[:, :, 0:SR],
                    cmp.rearrange("p (j m s) -> p m s j", j=6, m=G),
                    axis=AX.X, op=OP.add)

            # ---- slot-major gathers (3 tiles, 32-row padded molecules) --
            nstts, bsums = [], []
            for t, (m0, cnt) in enumerate(TILES3):
                rows = SJ * cnt
                nstt = ps_g.tile([96, FA], f32, tag="nstt", bufs=2)
                braw = ps_g.tile([96, D * FB], f32, tag="braw", bufs=1)
                for k in range(cnt):
                    m = m0 + k
                    rb = SJ * k
                    nc.tensor.matmul(nstt[rb:rb + SJ, :],
                                     et[:, m * SJ:(m + 1) * SJ],
                                     atoms_g[:, m * FA:(m + 1) * FA])
                    nc.tensor.matmul(
                        braw[rb:rb + SJ, :],
                        pmm[:, m * SJ:(m + 1) * SJ],
                        bonds_g[:, m * D * FB:(m + 1) * D * FB])
                nstt_sb = pmid.tile([96, FA], bf16, tag="nstts", bufs=3)
                nc.scalar.copy(nstt_sb[0:rows, :], nstt[0:rows, :])
                bsum = pmid.tile([96, FB], bf16, tag="bsum", bufs=3)
                with nc.allow_low_precision(reason="bf16 bond sums"):
                    nc.vector.tensor_reduce(
                        bsum[0:rows, :],
                        braw[0:rows, :].rearrange("p (d f) -> p f d", d=D),
                        axis=AX.X, op=OP.add)
                nstts.append((nstt_sb, rows))
                bsums.append(bsum)

            # ---- permute slot-major -> degree-block order ---------------
            for t in range(3):
                nc.tensor.matmul(f0p, nstts[t][0][0:nstts[t][1], 0:128],
                                 rts[t], start=(t == 0), stop=(t == 2))
            for t in range(3):
                nc.tensor.matmul(f1p, nstts[t][0][0:nstts[t][1], 128:256],
                                 rts[t], start=(t == 0), stop=(t == 2))
            for t in range(3):
                nc.tensor.matmul(f2p, bsums[t][0:nstts[t][1], :],
                                 rts[t], start=(t == 0), stop=(t == 2))
            f0 = pmid.tile([128, Q], bf16)
            nc.scalar.copy(f0[:], f0p)
            f1 = pmid.tile([128, Q], bf16)
            nc.scalar.copy(f1[:], f1p)
            f2 = pmid.tile([FB + 1, Q], bf16)
            nc.scalar.copy(f2[0:FB, :], f2p)
            nc.gpsimd.memset(f2[FB:FB + 1, :], 1.0)

            # ---- dense: deg4 q[0,96) w cols [C,2C); deg3 [96,128) -------
            for zt, q0, q1, c0 in ((z4, 0, Q3_BASE, C),
                                   (z3, Q3_BASE, Q, 0)):
                nc.tensor.matmul(zt, f0[:, q0:q1], w0[:, c0:c0 + C],
                                 start=True, stop=False)
                nc.tensor.matmul(zt, f1[:, q0:q1], w1[:, c0:c0 + C],
                                 start=False, stop=False)
                nc.tensor.matmul(zt, f2[:, q0:q1], w2[:, c0:c0 + C],
                                 start=False, stop=True)

            out_sb = pout.tile([Q, C], bf16)
            nc.scalar.activation(out_sb[0:Q3_BASE, :], z4, AF.Relu)
            nc.scalar.activation(out_sb[Q3_BASE:Q, :], z3, AF.Relu)
            nc.gpsimd.dma_start(out=out_ap[bg], in_=out_sb[:])

    nc.compile()
    return nc


def _get_nc():
    if "nc" not in _CACHE:
        _CACHE["nc"] = _build_program()
    return _CACHE["nc"]


def _make_in_maps(atoms, bonds, edges, W, b):
    import ml_dtypes

    bf16 = ml_dtypes.bfloat16
    atoms = np.asarray(atoms, dtype=np.float32)
    bonds = np.asarray(bonds, dtype=np.float32)
    edges = np.asarray(edges)
    W = np.asarray(W, dtype=np.float32)
    b = np.asarray(b, dtype=np.float32)

    # group-major layouts: (core, group, A, G*feat)
    def grp(x, feat):
        return np.ascontiguousarray(
            x.reshape(NCORES, NG, G, A, feat).transpose(0, 1, 3, 2, 4)
            .reshape(NCORES, NG, A, G * feat).astype(bf16))

    atoms_h = grp(atoms, FA)
    bonds_h = grp(bonds.reshape(B, A, D * FB), D * FB)
    edeg6 = np.concatenate(
        [edges.astype(np.float32),
         np.broadcast_to(np.arange(A, dtype=np.float32), (B, A))[..., None]],
        axis=-1)                                         # (B, A, 6)
    edeg_h = grp(edeg6, 6)

    # weights for degrees (3, 4): cols [0,C) = deg3, [C,2C) = deg4
    waug = np.concatenate([W, b[:, None, :]], axis=1)       # (5, 321, 256)
    w34 = waug[[3, 4]]                                       # (2, 321, 256)
    wpack = np.zeros((128, 6 * C), dtype=np.float32)
    wpack[:, 0:2 * C] = w34[:, 0:128].transpose(1, 0, 2).reshape(128, 2 * C)
    wpack[:, 2 * C:4 * C] = w34[:, 128:256].transpose(1, 0, 2).reshape(
        128, 2 * C)
    wpack[0:FB + 1, 4 * C:6 * C] = w34[:, 256:321].transpose(1, 0, 2).reshape(
        FB + 1, 2 * C)
    wpack = wpack.astype(bf16)

    cpack = np.zeros((A, C_TOT), dtype=np.float32)
    # ltri[k, m] = 1 if k <= m  (inclusive prefix sums via ltri^T @ mask)
    cpack[:, C_LTRI:C_LTRI + A] = np.triu(np.ones((A, A), dtype=np.float32))
    iotaj_row = np.zeros(G * SJ, dtype=np.float32)
    for m in range(G):
        for j in range(SR):
            iotaj_row[m * SJ + j] = (j + 1) if j < CAP4 else (j - CAP4 + 1)
    cpack[:, C_IOTAJ:C_IOTAJ + G * SJ] = iotaj_row
    cpack[:, C_LADDER:C_LADDER + 2 * G] = np.array(
        [3.0] * G + [4.0] * G, dtype=np.float32)
    for j in range(6):
        cpack[j, C_SEL + j * A:C_SEL + (j + 1) * A] = 1.0
    cpack = cpack.astype(bf16)

    rpack = np.zeros((96, 3 * Q), dtype=np.float32)
    for t, (m0, cnt) in enumerate(TILES3):
        for k in range(cnt):
            m = m0 + k
            for j in range(SR):
                q = (m * CAP4 + j) if j < CAP4 else (
                    Q3_BASE + m * CAP3 + (j - CAP4))
                rpack[SJ * k + j, t * Q + q] = 1.0
    rpack = rpack.astype(bf16)

    return [
        {
            "atoms": atoms_h[c],
            "bonds": bonds_h[c],
            "edeg": edeg_h[c],
            "cpack": cpack, "rpack": rpack, "wpack": wpack,
            "iotac": np.arange(A, dtype=np.float32).reshape(A, 1),
        }
        for c in range(NCORES)
    ]


def _assemble(out_hw, atoms, bonds, edges, W, b):
    """Scatter compact HW rows into the full output; numpy fallback for
    atoms outside the static slot capacities (deg<=2 or rank overflow)."""
    atoms = np.asarray(atoms, dtype=np.float32)
    bonds = np.asarray(bonds, dtype=np.float32)
    edges = np.asarray(edges)
    W = np.asarray(W, dtype=np.float32)
    b = np.asarray(b, dtype=np.float32)

    deg = (edges != -1).sum(-1)                         # (B, A)
    out = np.zeros((B, A, C), dtype=np.float32)
    covered = np.zeros((B, A), dtype=bool)
    gi = np.arange(B) // G                              # global group index
    ii = np.arange(B) % G                               # molecule in group

    for d, cap, base in ((4, CAP4, 0), (3, CAP3, Q3_BASE)):
        mask = deg == d
        rank = np.cumsum(mask, axis=1)
        ok = mask & (rank <= cap)
        mi, ai = np.nonzero(ok)
        q = base + ii[mi] * cap + (rank[mi, ai] - 1)
        out[mi, ai] = out_hw[gi[mi], q].astype(np.float32)
        covered |= ok

    rest = (deg < D) & ~covered
    for m, a in zip(*np.nonzero(rest)):
        e = edges[m, a]
        e = e[e >= 0]
        fa = atoms[m, a] + (atoms[m, e].sum(0) if e.size else 0.0)
        feat = np.concatenate([fa, bonds[m, a].sum(0)])
        z = feat @ W[deg[m, a]] + b[deg[m, a]]
        out[m, a] = np.maximum(z, 0.0)
    return out


def run_sharded(atoms, bonds, edges, W, b, trace=False):
    """Run on the 8 NeuronCores; returns (output, BassKernelResults)."""
    from concourse.bass_utils import run_bass_kernel_spmd

    nc = _get_nc()
    in_maps = _make_in_maps(atoms, bonds, edges, W, b)
    res = run_bass_kernel_spmd(nc, in_maps, list(range(NCORES)), trace=trace)
    out_hw = np.concatenate(
        [np.asarray(res.results[c]["out"]) for c in range(NCORES)],
        axis=0)                                          # (NCORES*NG, Q, C)
    out = _assemble(out_hw, atoms, bonds, edges, W, b)
    return out, res


def kernel(atoms, bonds, edges, W, b):
    out, _ = run_sharded(atoms, bonds, edges, W, b)
    return out


# revision 33
# speedup vs baseline: 1.6012x; 1.0264x over previous
"""Trainium2 Bass kernel for NeuralGraphHidden (GNN message passing).

Full-input contract: kernel(**inputs) takes the complete unsharded arrays,
shards batch dim 0 across 8 NeuronCores (data parallel), runs one SPMD Bass
program, and reassembles the full output.

Key structural fact exploited: deg[a] = #(edges[a,:] != -1) is in 0..5, but
the reference's degree mask covers only 0..4 - atoms with deg==5 (about 96%
of atoms for this input distribution) produce an all-zero output row.  The
kernel compacts the few deg<5 atoms per molecule into static per-degree
slots on-chip and runs the dense layer only on those slots:

  per group of 8 molecules (4 groups/core, 32 molecules/core):
    sumbond   = bonds pre-summed over the 5 bond slots by the
                DMA compute engine (accum_op=add)             (DMA CCE)
    deg       = row counts of edges != -1                     (DVE)
    rank_d    = per-degree prefix sums via tri-matmul          (PE)
    P         = slot one-hot (atom -> slot), 32-padded         (DVE)
    gatt      = per-slot edge ids via edeg^T @ P               (PE)
    E_j       = edge ids broadcast down partitions             (PE)
    ET        = sum_j onehot(E_j) + P  (neighbour+self)        (DVE)
    nstt|braw = [atoms | sumbond] gathered per slot            (PE)
    f0/f1/f2  = feats re-permuted slot->degree-block via
                constant permutation matmuls                   (PE)
    z4/z3     = per-degree dense with W_4 / W_3 (+bias row)    (PE)
    out       = relu(z)                                        (ScalarE)

  Matmul PSUM outputs may only start at partition 0/32/64, so per-molecule
  slot rows are 32-padded and grouped 3+3+2 molecules into three gather
  tiles; the dense stage uses two tiles (deg4: 96 rows, deg3: 32 rows).
  Slot capacities: 12 deg-4 + 4 deg-3 per molecule (measured maxima 12/2).
  deg<=2 atoms and any capacity overflow fall back to a tiny numpy path on
  the host (0-1 atoms in practice).  The host scatters the compact HW rows
  into the zero-initialised full output.
"""

import sys

sys.path.insert(0, "/opt/trn_rl_repo")

import numpy as np

B, A, D = 256, 128, 5
FA, FB, C = 256, 64, 256
NCORES = 8
BL = B // NCORES   # 32 molecules per core
G = 8              # molecules per group
NG = BL // G       # 4 groups per core

SJ = 32            # padded slots per molecule (16 real: 12 deg4 + 4 deg3)
SR = 16            # real slots per molecule
CAP4, CAP3 = 12, 4
Q = G * SR         # 128 dense-stage slots per group
Q3_BASE = G * CAP4  # 96: deg3 block starts here in q space
TILES3 = ((0, 3), (3, 3), (6, 2))   # (first molecule, count) per gather tile

# packed-const column offsets (cpack0, bf16, 128 partitions)
C_LTRI = 0          # (128, 128)
C_IOTAJ = 128       # (128, 256)
C_LADDER = 384      # (128, 16)
C0_TOT = 400

_CACHE = {}


def _build_program():
    from contextlib import ExitStack

    import concourse.bass as bass
    import concourse.tile as tile
    from concourse import bacc, mybir

    f32 = mybir.dt.float32
    bf16 = mybir.dt.bfloat16
    AF = mybir.ActivationFunctionType
    OP = mybir.AluOpType
    AX = mybir.AxisListType

    nc = bacc.Bacc("TRN2", target_bir_lowering=False, debug=False,
                   num_devices=NCORES)

    atoms_d = nc.dram_tensor("atoms", [NG, A, G * FA], bf16,
                             kind="ExternalInput")
    bonds_d = nc.dram_tensor("bonds", [NG, A, G * D * FB], bf16,
                             kind="ExternalInput")
    edeg_d = nc.dram_tensor("edeg", [NG, A, G * 6], bf16,
                            kind="ExternalInput")
    cpack_d = nc.dram_tensor("cpack", [A, C0_TOT], bf16,
                             kind="ExternalInput")
    selc_d = nc.dram_tensor("selc", [6, 6 * A], bf16, kind="ExternalInput")
    iotac_d = nc.dram_tensor("iotac", [A, 1], f32, kind="ExternalInput")
    rpack_d = nc.dram_tensor("rpack", [96, 3 * Q], bf16,
                             kind="ExternalInput")
    wpack_d = nc.dram_tensor("wpack", [128, 3 * 2 * C], bf16,
                             kind="ExternalInput")
    out_d = nc.dram_tensor("out", [NG, Q, C], bf16, kind="ExternalOutput")

    atoms_ap = atoms_d.ap()
    bonds_ap = bonds_d.ap()
    edeg_ap = edeg_d.ap()
    out_ap = out_d.ap()

    with tile.TileContext(nc) as tc, ExitStack() as ctx:
        consts = ctx.enter_context(tc.tile_pool(name="consts", bufs=1))
        pin = ctx.enter_context(tc.tile_pool(name="pin", bufs=3))
        pmid = ctx.enter_context(tc.tile_pool(name="pmid", bufs=3))
        pout = ctx.enter_context(tc.tile_pool(name="pout", bufs=4))
        ps_e = ctx.enter_context(
            tc.tile_pool(name="ps_e", bufs=1, space="PSUM"))
        ps_g = ctx.enter_context(
            tc.tile_pool(name="ps_g", bufs=1, space="PSUM"))

        # ---- packed constants: 3 DMAs total -----------------------------
        cpack = consts.tile([A, C0_TOT], bf16)
        nc.scalar.dma_start(out=cpack[:], in_=cpack_d.ap()[:])
        selc = consts.tile([6, 6 * A], bf16)
        nc.scalar.dma_start(out=selc[:], in_=selc_d.ap()[:])
        rpack = consts.tile([96, 3 * Q], bf16)
        nc.scalar.dma_start(out=rpack[:], in_=rpack_d.ap()[:])
        wpack = consts.tile([128, 3 * 2 * C], bf16)
        nc.scalar.dma_start(out=wpack[:], in_=wpack_d.ap()[:])
        iotac_t = consts.tile([A, 1], f32)
        nc.scalar.dma_start(out=iotac_t[:], in_=iotac_d.ap()[:])

        ltri = cpack[:, C_LTRI:C_LTRI + 128]
        iotaj = cpack[:, C_IOTAJ:C_IOTAJ + G * SJ]
        ladder = cpack[:, C_LADDER:C_LADDER + 2 * G]
        iotac = iotac_t[:]
        sel = selc[:]
        rts = [rpack[0:96, 0:Q], rpack[0:96, Q:2 * Q],
               rpack[0:64, 2 * Q:3 * Q]]
        w0 = wpack[:, 0:2 * C]
        w1 = wpack[:, 2 * C:4 * C]
        w2 = wpack[0:FB + 1, 4 * C:6 * C]

        for bg in range(NG):
            # ---- inputs; bonds pre-summed over d by DMA CCE -------------
            edeg_g = pin.tile([A, G * 6], bf16)
            nc.sync.dma_start(out=edeg_g[:], in_=edeg_ap[bg])
            atoms_g = pin.tile([A, G * FA], bf16)
            nc.sync.dma_start(out=atoms_g[:], in_=atoms_ap[bg])
            bonds_g = pin.tile([A, G * D * FB], bf16)
            nc.sync.dma_start(out=bonds_g[:], in_=bonds_ap[bg])

            # PSUM bank packing (8 banks of 2KB):
            #   early x2, e04 x1, zf x2, nstt x2, braw x1
            early = ps_e.tile([A, 400], f32, tag="early", bufs=2)
            pos2 = early[:, 0:16]
            gatt = early[0:6, 16:144]
            e45 = early[:, 144:400]
            e04 = ps_e.tile([A, 4 * Q], f32, tag="e04", bufs=1)
            zf = ps_e.tile([A, 512], f32, tag="zf", bufs=2)
            f0p = zf[:, 0:Q]
            f1p = zf[:, Q:2 * Q]
            f2p = zf[0:FB, 2 * Q:3 * Q]
            z4 = zf[0:Q3_BASE, 0:C]
            z3 = zf[0:Q - Q3_BASE, C:2 * C]

            # ---- degree + per-degree ranks ------------------------------
            ne = pmid.tile([A, G * D], bf16)
            nc.vector.tensor_scalar(
                ne.rearrange("p (g d) -> p g d", g=G),
                edeg_g.rearrange("p (g d) -> p g d", g=G)[:, :, 0:D],
                -1.0, None, OP.not_equal)
            deg = pmid.tile([A, G], bf16)
            with nc.allow_low_precision(reason="counts <= 5 exact in bf16"):
                nc.vector.tensor_reduce(
                    deg[:], ne.rearrange("p (g d) -> p g d", g=G),
                    axis=AX.X, op=OP.add)
            masks2 = pmid.tile([A, 2 * G], bf16)
            nc.vector.tensor_tensor(
                masks2.rearrange("p (d g) -> p d g", d=2),
                ladder.rearrange("p (d g) -> p d g", d=2),
                deg.unsqueeze(1).broadcast_to((A, 2, G)),
                OP.is_equal)
            nc.tensor.matmul(pos2, ltri, masks2[:])
            posm2 = pmid.tile([A, 2 * G], bf16)
            with nc.allow_low_precision(reason="ranks <= 128 exact in bf16"):
                nc.vector.tensor_tensor(posm2[:], pos2, masks2[:], OP.mult)

            # ---- slot one-hot P, 32-padded per molecule -----------------
            pmm = pmid.tile([A, G * SJ], bf16)
            nc.gpsimd.memset(pmm[:], 0.0)
            pv = pmm.rearrange("p (m j) -> p m j", m=G)
            iv = iotaj.rearrange("p (m j) -> p m j", m=G)
            nc.vector.tensor_tensor(
                pv[:, :, 0:CAP4], iv[:, :, 0:CAP4],
                posm2[:, G:2 * G].unsqueeze(2).broadcast_to((A, G, CAP4)),
                OP.is_equal)
            nc.vector.tensor_tensor(
                pv[:, :, CAP4:SR], iv[:, :, CAP4:SR],
                posm2[:, 0:G].unsqueeze(2).broadcast_to((A, G, CAP3)),
                OP.is_equal)

            # ---- per-slot edge ids, broadcast, onehot sum ---------------
            for m in range(G):
                nc.tensor.matmul(gatt[:, m * SR:(m + 1) * SR],
                                 edeg_g[:, m * 6:(m + 1) * 6],
                                 pmm[:, m * SJ:m * SJ + SR])
            gatt_sb = pmid.tile([6, Q], bf16)
            nc.scalar.copy(gatt_sb[:], gatt)

            for j in range(4):
                nc.tensor.matmul(e04[:, j * Q:(j + 1) * Q],
                                 sel[:, j * A:(j + 1) * A], gatt_sb[:])
            for j in (4, 5):
                nc.tensor.matmul(e45[:, (j - 4) * Q:(j - 3) * Q],
                                 sel[:, j * A:(j + 1) * A], gatt_sb[:])

            cmp = pmid.tile([A, 6 * Q], bf16)
            nc.vector.tensor_scalar(cmp[:, 0:4 * Q], e04[:], iotac, None,
                                    OP.is_equal)
            nc.vector.tensor_scalar(cmp[:, 4 * Q:6 * Q], e45, iotac,
                                    None, OP.is_equal)
            et = pmid.tile([A, G * SJ], bf16)
            nc.gpsimd.memset(et[:], 0.0)
            ev = et.rearrange("p (m j) -> p m j", m=G)
            with nc.allow_low_precision(reason="counts <= 6 exact in bf16"):
                nc.vector.tensor_reduce(
                    ev[:, :, 0:SR],
                    cmp.rearrange("p (j m s) -> p m s j", j=6, m=G),
                    axis=AX.X, op=OP.add)

            # ---- slot-major gathers (3 tiles, 32-row padded molecules) --
            nstts, bsums = [], []
            for t, (m0, cnt) in enumerate(TILES3):
                rows = SJ * cnt
                nstt = ps_g.tile([96, FA], f32, tag="nstt", bufs=2)
                braw = ps_g.tile([96, D * FB], f32, tag="braw", bufs=1)
                for k in range(cnt):
                    m = m0 + k
                    rb = SJ * k
                    nc.tensor.matmul(nstt[rb:rb + SJ, :],
                                     et[:, m * SJ:(m + 1) * SJ],
                                     atoms_g[:, m * FA:(m + 1) * FA])
                    nc.tensor.matmul(
                        braw[rb:rb + SJ, :],
                        pmm[:, m * SJ:(m + 1) * SJ],
                        bonds_g[:, m * D * FB:(m + 1) * D * FB])
                nstt_sb = pmid.tile([96, FA], bf16, tag="nstts", bufs=3)
                nc.scalar.copy(nstt_sb[0:rows, :], nstt[0:rows, :])
                bsum = pmid.tile([96, FB], bf16, tag="bsum", bufs=3)
                with nc.allow_low_precision(reason="bf16 bond sums"):
                    nc.vector.tensor_reduce(
                        bsum[0:rows, :],
                        braw[0:rows, :].rearrange("p (d f) -> p f d", d=D),
                        axis=AX.X, op=OP.add)
                nstts.append((nstt_sb, rows))
                bsums.append(bsum)

            # ---- permute slot-major -> degree-block order ---------------
            for t in range(3):
                nc.tensor.matmul(f0p, nstts[t][0][0:nstts[t][1], 0:128],
                                 rts[t], start=(t == 0), stop=(t == 2))
            for t in range(3):
                nc.tensor.matmul(f1p, nstts[t][0][0:nstts[t][1], 128:256],
                                 rts[t], start=(t == 0), stop=(t == 2))
            for t in range(3):
                nc.tensor.matmul(f2p, bsums[t][0:nstts[t][1], :],
                                 rts[t], start=(t == 0), stop=(t == 2))
            f0 = pmid.tile([128, Q], bf16)
            nc.scalar.copy(f0[:], f0p)
            f1 = pmid.tile([128, Q], bf16)
            nc.scalar.copy(f1[:], f1p)
            f2 = pmid.tile([FB + 1, Q], bf16)
            nc.scalar.copy(f2[0:FB, :], f2p)
            nc.gpsimd.memset(f2[FB:FB + 1, :], 1.0)

            # ---- dense: deg4 q[0,96) w cols [C,2C); deg3 [96,128) -------
            for zt, q0, q1, c0 in ((z4, 0, Q3_BASE, C),
                                   (z3, Q3_BASE, Q, 0)):
                nc.tensor.matmul(zt, f0[:, q0:q1], w0[:, c0:c0 + C],
                                 start=True, stop=False)
                nc.tensor.matmul(zt, f1[:, q0:q1], w1[:, c0:c0 + C],
                                 start=False, stop=False)
                nc.tensor.matmul(zt, f2[:, q0:q1], w2[:, c0:c0 + C],
                                 start=False, stop=True)

            out_sb = pout.tile([Q, C], bf16)
            nc.scalar.activation(out_sb[0:Q3_BASE, :], z4, AF.Relu)
            nc.scalar.activation(out_sb[Q3_BASE:Q, :], z3, AF.Relu)
            nc.gpsimd.dma_start(out=out_ap[bg], in_=out_sb[:])

    nc.compile()
    return nc


def _get_nc():
    if "nc" not in _CACHE:
        _CACHE["nc"] = _build_program()
    return _CACHE["nc"]


def _make_in_maps(atoms, bonds, edges, W, b):
    import ml_dtypes

    bf16 = ml_dtypes.bfloat16
    atoms = np.asarray(atoms, dtype=np.float32)
    bonds = np.asarray(bonds, dtype=np.float32)
    edges = np.asarray(edges)
    W = np.asarray(W, dtype=np.float32)
    b = np.asarray(b, dtype=np.float32)

    # group-major layouts: (core, group, A, G*feat)
    def grp(x, feat):
        return np.ascontiguousarray(
            x.reshape(NCORES, NG, G, A, feat).transpose(0, 1, 3, 2, 4)
            .reshape(NCORES, NG, A, G * feat).astype(bf16))

    atoms_h = grp(atoms, FA)
    bonds_h = grp(bonds.reshape(B, A, D * FB), D * FB)
    edeg6 = np.concatenate(
        [edges.astype(np.float32),
         np.broadcast_to(np.arange(A, dtype=np.float32), (B, A))[..., None]],
        axis=-1)                                         # (B, A, 6)
    edeg_h = grp(edeg6, 6)

    # weights for degrees (3, 4): cols [0,C) = deg3, [C,2C) = deg4
    waug = np.concatenate([W, b[:, None, :]], axis=1)       # (5, 321, 256)
    w34 = waug[[3, 4]]                                       # (2, 321, 256)
    wpack = np.zeros((128, 6 * C), dtype=np.float32)
    wpack[:, 0:2 * C] = w34[:, 0:128].transpose(1, 0, 2).reshape(128, 2 * C)
    wpack[:, 2 * C:4 * C] = w34[:, 128:256].transpose(1, 0, 2).reshape(
        128, 2 * C)
    wpack[0:FB + 1, 4 * C:6 * C] = w34[:, 256:321].transpose(1, 0, 2).reshape(
        FB + 1, 2 * C)
    wpack = wpack.astype(bf16)

    cpack = np.zeros((A, C0_TOT), dtype=np.float32)
    # ltri[k, m] = 1 if k <= m  (inclusive prefix sums via ltri^T @ mask)
    cpack[:, C_LTRI:C_LTRI + A] = np.triu(np.ones((A, A), dtype=np.float32))
    iotaj_row = np.zeros(G * SJ, dtype=np.float32)
    for m in range(G):
        for j in range(SR):
            iotaj_row[m * SJ + j] = (j + 1) if j < CAP4 else (j - CAP4 + 1)
    cpack[:, C_IOTAJ:C_IOTAJ + G * SJ] = iotaj_row
    cpack[:, C_LADDER:C_LADDER + 2 * G] = np.array(
        [3.0] * G + [4.0] * G, dtype=np.float32)
    cpack = cpack.astype(bf16)
    selc = np.zeros((6, 6 * A), dtype=np.float32)
    for j in range(6):
        selc[j, j * A:(j + 1) * A] = 1.0
    selc = selc.astype(bf16)

    rpack = np.zeros((96, 3 * Q), dtype=np.float32)
    for t, (m0, cnt) in enumerate(TILES3):
        for k in range(cnt):
            m = m0 + k
            for j in range(SR):
                q = (m * CAP4 + j) if j < CAP4 else (
                    Q3_BASE + m * CAP3 + (j - CAP4))
                rpack[SJ * k + j, t * Q + q] = 1.0
    rpack = rpack.astype(bf16)

    return [
        {
            "atoms": atoms_h[c],
            "bonds": bonds_h[c],
            "edeg": edeg_h[c],
            "cpack": cpack, "selc": selc, "rpack": rpack, "wpack": wpack,
            "iotac": np.arange(A, dtype=np.float32).reshape(A, 1),
        }
        for c in range(NCORES)
    ]


def _assemble(out_hw, atoms, bonds, edges, W, b):
    """Scatter compact HW rows into the full output; numpy fallback for
    atoms outside the static slot capacities (deg<=2 or rank overflow)."""
    atoms = np.asarray(atoms, dtype=np.float32)
    bonds = np.asarray(bonds, dtype=np.float32)
    edges = np.asarray(edges)
    W = np.asarray(W, dtype=np.float32)
    b = np.asarray(b, dtype=np.float32)

    deg = (edges != -1).sum(-1)                         # (B, A)
    out = np.zeros((B, A, C), dtype=np.float32)
    covered = np.zeros((B, A), dtype=bool)
    gi = np.arange(B) // G                              # global group index
    ii = np.arange(B) % G                               # molecule in group

    for d, cap, base in ((4, CAP4, 0), (3, CAP3, Q3_BASE)):
        mask = deg == d
        rank = np.cumsum(mask, axis=1)
        ok = mask & (rank <= cap)
        mi, ai = np.nonzero(ok)
        q = base + ii[mi] * cap + (rank[mi, ai] - 1)
        out[mi, ai] = out_hw[gi[mi], q].astype(np.float32)
        covered |= ok

    rest = (deg < D) & ~covered
    for m, a in zip(*np.nonzero(rest)):
        e = edges[m, a]
        e = e[e >= 0]
        fa = atoms[m, a] + (atoms[m, e].sum(0) if e.size else 0.0)
        feat = np.concatenate([fa, bonds[m, a].sum(0)])
        z = feat @ W[deg[m, a]] + b[deg[m, a]]
        out[m, a] = np.maximum(z, 0.0)
    return out


def run_sharded(atoms, bonds, edges, W, b, trace=False):
    """Run on the 8 NeuronCores; returns (output, BassKernelResults)."""
    from concourse.bass_utils import run_bass_kernel_spmd

    nc = _get_nc()
    in_maps = _make_in_maps(atoms, bonds, edges, W, b)
    res = run_bass_kernel_spmd(nc, in_maps, list(range(NCORES)), trace=trace)
    out_hw = np.concatenate(
        [np.asarray(res.results[c]["out"]) for c in range(NCORES)],
        axis=0)                                          # (NCORES*NG, Q, C)
    out = _assemble(out_hw, atoms, bonds, edges, W, b)
    return out, res


def kernel(atoms, bonds, edges, W, b):
    out, _ = run_sharded(atoms, bonds, edges, W, b)
    return out


# revision 34
# speedup vs baseline: 1.6549x; 1.0335x over previous
"""Trainium2 Bass kernel for NeuralGraphHidden (GNN message passing).

Full-input contract: kernel(**inputs) takes the complete unsharded arrays,
shards batch dim 0 across 8 NeuronCores (data parallel), runs one SPMD Bass
program, and reassembles the full output.

Key structural fact exploited: deg[a] = #(edges[a,:] != -1) is in 0..5, but
the reference's degree mask covers only 0..4 - atoms with deg==5 (about 96%
of atoms for this input distribution) produce an all-zero output row.  The
kernel compacts the few deg<5 atoms per molecule into static per-degree
slots on-chip and runs the dense layer only on those slots:

  per group of 8 molecules (4 groups/core, 32 molecules/core):
    sumbond   = bonds pre-summed over the 5 bond slots by the
                DMA compute engine (accum_op=add)             (DMA CCE)
    deg       = row counts of edges != -1                     (DVE)
    rank_d    = per-degree prefix sums via tri-matmul          (PE)
    P         = slot one-hot (atom -> slot), 32-padded         (DVE)
    gatt      = per-slot edge ids via edeg^T @ P               (PE)
    E_j       = edge ids broadcast down partitions             (PE)
    ET        = sum_j onehot(E_j) + P  (neighbour+self)        (DVE)
    nstt|braw = [atoms | sumbond] gathered per slot            (PE)
    f0/f1/f2  = feats re-permuted slot->degree-block via
                constant permutation matmuls                   (PE)
    z4/z3     = per-degree dense with W_4 / W_3 (+bias row)    (PE)
    out       = relu(z)                                        (ScalarE)

  Matmul PSUM outputs may only start at partition 0/32/64, so per-molecule
  slot rows are 32-padded and grouped 3+3+2 molecules into three gather
  tiles; the dense stage uses two tiles (deg4: 96 rows, deg3: 32 rows).
  Slot capacities: 12 deg-4 + 4 deg-3 per molecule (measured maxima 12/2).
  deg<=2 atoms and any capacity overflow fall back to a tiny numpy path on
  the host (0-1 atoms in practice).  The host scatters the compact HW rows
  into the zero-initialised full output.
"""

import sys

sys.path.insert(0, "/opt/trn_rl_repo")

import numpy as np

B, A, D = 256, 128, 5
FA, FB, C = 256, 64, 256
NCORES = 8
BL = B // NCORES   # 32 molecules per core
G = 8              # molecules per group
NG = BL // G       # 4 groups per core

SJ = 32            # padded slots per molecule (16 real: 12 deg4 + 4 deg3)
SR = 16            # real slots per molecule
CAP4, CAP3 = 12, 4
Q = G * SR         # 128 dense-stage slots per group
Q3_BASE = G * CAP4  # 96: deg3 block starts here in q space
TILES3 = ((0, 3), (3, 3), (6, 2))   # (first molecule, count) per gather tile

# packed-const column offsets (cpack0, bf16, 128 partitions)
C_LTRI = 0          # (128, 128)
C_IOTAJ = 128       # (128, 256)
C_LADDER = 384      # (128, 16)
C0_TOT = 400

_CACHE = {}


def _build_program():
    from contextlib import ExitStack

    import concourse.bass as bass
    import concourse.tile as tile
    from concourse import bacc, mybir

    f32 = mybir.dt.float32
    bf16 = mybir.dt.bfloat16
    AF = mybir.ActivationFunctionType
    OP = mybir.AluOpType
    AX = mybir.AxisListType

    nc = bacc.Bacc("TRN2", target_bir_lowering=False, debug=False,
                   num_devices=NCORES)

    atoms_d = nc.dram_tensor("atoms", [NG, A, G * FA], bf16,
                             kind="ExternalInput")
    bonds_d = nc.dram_tensor("bonds", [NG, A, G * D * FB], bf16,
                             kind="ExternalInput")
    edeg_d = nc.dram_tensor("edeg", [NG, A, G * 6], bf16,
                            kind="ExternalInput")
    cpack_d = nc.dram_tensor("cpack", [A, C0_TOT], bf16,
                             kind="ExternalInput")
    selc_d = nc.dram_tensor("selc", [6, 6 * A], bf16, kind="ExternalInput")
    iotac_d = nc.dram_tensor("iotac", [A, 1], f32, kind="ExternalInput")
    rpack_d = nc.dram_tensor("rpack", [96, 3 * Q], bf16,
                             kind="ExternalInput")
    wpack_d = nc.dram_tensor("wpack", [128, 3 * 2 * C], bf16,
                             kind="ExternalInput")
    out_d = nc.dram_tensor("out", [NG, Q, C], bf16, kind="ExternalOutput")

    atoms_ap = atoms_d.ap()
    bonds_ap = bonds_d.ap()
    edeg_ap = edeg_d.ap()
    out_ap = out_d.ap()

    with tile.TileContext(nc) as tc, ExitStack() as ctx:
        consts = ctx.enter_context(tc.tile_pool(name="consts", bufs=1))
        pin = ctx.enter_context(tc.tile_pool(name="pin", bufs=3))
        pmid = ctx.enter_context(tc.tile_pool(name="pmid", bufs=3))
        pout = ctx.enter_context(tc.tile_pool(name="pout", bufs=4))
        ps_e = ctx.enter_context(
            tc.tile_pool(name="ps_e", bufs=1, space="PSUM"))
        ps_g = ctx.enter_context(
            tc.tile_pool(name="ps_g", bufs=1, space="PSUM"))

        # ---- packed constants: 3 DMAs total -----------------------------
        cpack = consts.tile([A, C0_TOT], bf16)
        nc.scalar.dma_start(out=cpack[:], in_=cpack_d.ap()[:])
        selc = consts.tile([6, 6 * A], bf16)
        nc.scalar.dma_start(out=selc[:], in_=selc_d.ap()[:])
        rpack = consts.tile([96, 3 * Q], bf16)
        nc.scalar.dma_start(out=rpack[:], in_=rpack_d.ap()[:])
        wpack = consts.tile([128, 3 * 2 * C], bf16)
        nc.scalar.dma_start(out=wpack[:], in_=wpack_d.ap()[:])
        iotac_t = consts.tile([A, 1], f32)
        nc.scalar.dma_start(out=iotac_t[:], in_=iotac_d.ap()[:])

        ltri = cpack[:, C_LTRI:C_LTRI + 128]
        iotaj = cpack[:, C_IOTAJ:C_IOTAJ + G * SJ]
        ladder = cpack[:, C_LADDER:C_LADDER + 2 * G]
        iotac = iotac_t[:]
        sel = selc[:]
        rts = [rpack[0:96, 0:Q], rpack[0:96, Q:2 * Q],
               rpack[0:64, 2 * Q:3 * Q]]
        w0 = wpack[:, 0:2 * C]
        w1 = wpack[:, 2 * C:4 * C]
        w2 = wpack[0:FB + 1, 4 * C:6 * C]

        for bg in range(NG):
            # ---- inputs; bonds pre-summed over d by DMA CCE -------------
            edeg_g = pin.tile([A, G * 6], bf16)
            nc.sync.dma_start(out=edeg_g[:], in_=edeg_ap[bg])
            atoms_g = pin.tile([A, G * FA], bf16)
            nc.sync.dma_start(out=atoms_g[:], in_=atoms_ap[bg])
            bonds_g = pin.tile([A, G * D * FB], bf16)
            nc.sync.dma_start(out=bonds_g[:], in_=bonds_ap[bg])

            # PSUM bank packing (8 banks of 2KB):
            #   early x2, e04 x1, zf x2, nstt x2, braw x1
            early = ps_e.tile([A, 400], f32, tag="early", bufs=2)
            pos2 = early[:, 0:16]
            gatt = early[0:6, 16:144]
            e45 = early[:, 144:400]
            e04 = ps_e.tile([A, 4 * Q], f32, tag="e04", bufs=1)
            zf = ps_e.tile([A, 512], f32, tag="zf", bufs=2)
            z4 = zf[0:Q3_BASE, 0:C]
            z3 = zf[0:Q - Q3_BASE, C:2 * C]
            nf = ps_e.tile([A, 2 * Q], f32, tag="nf", bufs=2)

            # ---- degree + per-degree ranks ------------------------------
            ne = pmid.tile([A, G * D], bf16)
            nc.vector.tensor_scalar(
                ne.rearrange("p (g d) -> p g d", g=G),
                edeg_g.rearrange("p (g d) -> p g d", g=G)[:, :, 0:D],
                -1.0, None, OP.not_equal)
            deg = pmid.tile([A, G], bf16)
            with nc.allow_low_precision(reason="counts <= 5 exact in bf16"):
                nc.vector.tensor_reduce(
                    deg[:], ne.rearrange("p (g d) -> p g d", g=G),
                    axis=AX.X, op=OP.add)
            masks2 = pmid.tile([A, 2 * G], bf16)
            nc.vector.tensor_tensor(
                masks2.rearrange("p (d g) -> p d g", d=2),
                ladder.rearrange("p (d g) -> p d g", d=2),
                deg.unsqueeze(1).broadcast_to((A, 2, G)),
                OP.is_equal)
            nc.tensor.matmul(pos2, ltri, masks2[:])
            posm2 = pmid.tile([A, 2 * G], bf16)
            with nc.allow_low_precision(reason="ranks <= 128 exact in bf16"):
                nc.vector.tensor_tensor(posm2[:], pos2, masks2[:], OP.mult)

            # ---- slot one-hot P, 32-padded per molecule -----------------
            pmm = pmid.tile([A, G * SJ], bf16)
            nc.gpsimd.memset(pmm[:], 0.0)
            pv = pmm.rearrange("p (m j) -> p m j", m=G)
            iv = iotaj.rearrange("p (m j) -> p m j", m=G)
            nc.vector.tensor_tensor(
                pv[:, :, 0:CAP4], iv[:, :, 0:CAP4],
                posm2[:, G:2 * G].unsqueeze(2).broadcast_to((A, G, CAP4)),
                OP.is_equal)
            nc.vector.tensor_tensor(
                pv[:, :, CAP4:SR], iv[:, :, CAP4:SR],
                posm2[:, 0:G].unsqueeze(2).broadcast_to((A, G, CAP3)),
                OP.is_equal)

            # ---- per-slot edge ids, broadcast, onehot sum ---------------
            for m in range(G):
                nc.tensor.matmul(gatt[:, m * SR:(m + 1) * SR],
                                 edeg_g[:, m * 6:(m + 1) * 6],
                                 pmm[:, m * SJ:m * SJ + SR])
            gatt_sb = pmid.tile([6, Q], bf16)
            nc.scalar.copy(gatt_sb[:], gatt)

            for j in range(4):
                nc.tensor.matmul(e04[:, j * Q:(j + 1) * Q],
                                 sel[:, j * A:(j + 1) * A], gatt_sb[:])
            for j in (4, 5):
                nc.tensor.matmul(e45[:, (j - 4) * Q:(j - 3) * Q],
                                 sel[:, j * A:(j + 1) * A], gatt_sb[:])

            cmp = pmid.tile([A, 6 * Q], bf16)
            nc.vector.tensor_scalar(cmp[:, 0:4 * Q], e04[:], iotac, None,
                                    OP.is_equal)
            nc.vector.tensor_scalar(cmp[:, 4 * Q:6 * Q], e45, iotac,
                                    None, OP.is_equal)
            et = pmid.tile([A, Q], bf16)
            with nc.allow_low_precision(reason="counts <= 6 exact in bf16"):
                nc.vector.tensor_reduce(
                    et[:], cmp.rearrange("p (j q) -> p q j", j=6),
                    axis=AX.X, op=OP.add)

            # ---- gathers: atoms feat-major (nf), bonds slot-major -------
            for m in range(G):
                for k in range(2):
                    nc.tensor.matmul(
                        nf[:, k * Q + m * SR:k * Q + (m + 1) * SR],
                        atoms_g[:, m * FA + k * 128:m * FA + (k + 1) * 128],
                        et[:, m * SR:(m + 1) * SR])
            bsums = []
            for t, (m0, cnt) in enumerate(TILES3):
                rows = SJ * cnt
                braw = ps_g.tile([96, D * FB + 128], f32, tag="braw",
                                 bufs=1)
                for k in range(cnt):
                    m = m0 + k
                    rb = SJ * k
                    nc.tensor.matmul(
                        braw[rb:rb + SJ, 0:D * FB],
                        pmm[:, m * SJ:(m + 1) * SJ],
                        bonds_g[:, m * D * FB:(m + 1) * D * FB])
                bsum = pmid.tile([96, FB], bf16, tag="bsum", bufs=3)
                with nc.allow_low_precision(reason="bf16 bond sums"):
                    nc.vector.tensor_reduce(
                        bsum[0:rows, :],
                        braw[0:rows, 0:D * FB].rearrange(
                            "p (d f) -> p f d", d=D),
                        axis=AX.X, op=OP.add)
                bsums.append((bsum, rows))
            f2p = braw[0:FB, D * FB:D * FB + Q]
            for t in range(3):
                nc.tensor.matmul(f2p, bsums[t][0][0:bsums[t][1], :],
                                 rts[t], start=(t == 0), stop=(t == 2))

            # ---- permute slot-major -> degree-block via strided copies --
            f0 = pmid.tile([128, Q], bf16)
            f1 = pmid.tile([128, Q], bf16)
            for k, ft in ((0, f0), (1, f1)):
                nfv = nf[:, k * Q:(k + 1) * Q].rearrange(
                    "p (m s) -> p m s", m=G)
                nc.scalar.copy(
                    ft[:, 0:Q3_BASE].rearrange("p (m j) -> p m j", m=G),
                    nfv[:, :, 0:CAP4])
                nc.scalar.copy(
                    ft[:, Q3_BASE:Q].rearrange("p (m j) -> p m j", m=G),
                    nfv[:, :, CAP4:SR])
            f2 = pmid.tile([FB + 1, Q], bf16)
            nc.scalar.copy(f2[0:FB, :], f2p)
            nc.gpsimd.memset(f2[FB:FB + 1, :], 1.0)

            # ---- dense: deg4 q[0,96) w cols [C,2C); deg3 [96,128) -------
            for zt, q0, q1, c0 in ((z4, 0, Q3_BASE, C),
                                   (z3, Q3_BASE, Q, 0)):
                nc.tensor.matmul(zt, f0[:, q0:q1], w0[:, c0:c0 + C],
                                 start=True, stop=False)
                nc.tensor.matmul(zt, f1[:, q0:q1], w1[:, c0:c0 + C],
                                 start=False, stop=False)
                nc.tensor.matmul(zt, f2[:, q0:q1], w2[:, c0:c0 + C],
                                 start=False, stop=True)

            out_sb = pout.tile([Q, C], bf16)
            nc.scalar.activation(out_sb[0:Q3_BASE, :], z4, AF.Relu)
            nc.scalar.activation(out_sb[Q3_BASE:Q, :], z3, AF.Relu)
            nc.gpsimd.dma_start(out=out_ap[bg], in_=out_sb[:])

    nc.compile()
    return nc


def _get_nc():
    if "nc" not in _CACHE:
        _CACHE["nc"] = _build_program()
    return _CACHE["nc"]


def _make_in_maps(atoms, bonds, edges, W, b):
    import ml_dtypes

    bf16 = ml_dtypes.bfloat16
    atoms = np.asarray(atoms, dtype=np.float32)
    bonds = np.asarray(bonds, dtype=np.float32)
    edges = np.asarray(edges)
    W = np.asarray(W, dtype=np.float32)
    b = np.asarray(b, dtype=np.float32)

    # group-major layouts: (core, group, A, G*feat)
    def grp(x, feat):
        return np.ascontiguousarray(
            x.reshape(NCORES, NG, G, A, feat).transpose(0, 1, 3, 2, 4)
            .reshape(NCORES, NG, A, G * feat).astype(bf16))

    atoms_h = grp(atoms, FA)
    bonds_h = grp(bonds.reshape(B, A, D * FB), D * FB)
    edeg6 = np.concatenate(
        [edges.astype(np.float32),
         np.broadcast_to(np.arange(A, dtype=np.float32), (B, A))[..., None]],
        axis=-1)                                         # (B, A, 6)
    edeg_h = grp(edeg6, 6)

    # weights for degrees (3, 4): cols [0,C) = deg3, [C,2C) = deg4
    waug = np.concatenate([W, b[:, None, :]], axis=1)       # (5, 321, 256)
    w34 = waug[[3, 4]]                                       # (2, 321, 256)
    wpack = np.zeros((128, 6 * C), dtype=np.float32)
    wpack[:, 0:2 * C] = w34[:, 0:128].transpose(1, 0, 2).reshape(128, 2 * C)
    wpack[:, 2 * C:4 * C] = w34[:, 128:256].transpose(1, 0, 2).reshape(
        128, 2 * C)
    wpack[0:FB + 1, 4 * C:6 * C] = w34[:, 256:321].transpose(1, 0, 2).reshape(
        FB + 1, 2 * C)
    wpack = wpack.astype(bf16)

    cpack = np.zeros((A, C0_TOT), dtype=np.float32)
    # ltri[k, m] = 1 if k <= m  (inclusive prefix sums via ltri^T @ mask)
    cpack[:, C_LTRI:C_LTRI + A] = np.triu(np.ones((A, A), dtype=np.float32))
    iotaj_row = np.zeros(G * SJ, dtype=np.float32)
    for m in range(G):
        for j in range(SR):
            iotaj_row[m * SJ + j] = (j + 1) if j < CAP4 else (j - CAP4 + 1)
    cpack[:, C_IOTAJ:C_IOTAJ + G * SJ] = iotaj_row
    cpack[:, C_LADDER:C_LADDER + 2 * G] = np.array(
        [3.0] * G + [4.0] * G, dtype=np.float32)
    cpack = cpack.astype(bf16)
    selc = np.zeros((6, 6 * A), dtype=np.float32)
    for j in range(6):
        selc[j, j * A:(j + 1) * A] = 1.0
    selc = selc.astype(bf16)

    rpack = np.zeros((96, 3 * Q), dtype=np.float32)
    for t, (m0, cnt) in enumerate(TILES3):
        for k in range(cnt):
            m = m0 + k
            for j in range(SR):
                q = (m * CAP4 + j) if j < CAP4 else (
                    Q3_BASE + m * CAP3 + (j - CAP4))
                rpack[SJ * k + j, t * Q + q] = 1.0
    rpack = rpack.astype(bf16)

    return [
        {
            "atoms": atoms_h[c],
            "bonds": bonds_h[c],
            "edeg": edeg_h[c],
            "cpack": cpack, "selc": selc, "rpack": rpack, "wpack": wpack,
            "iotac": np.arange(A, dtype=np.float32).reshape(A, 1),
        }
        for c in range(NCORES)
    ]


def _assemble(out_hw, atoms, bonds, edges, W, b):
    """Scatter compact HW rows into the full output; numpy fallback for
    atoms outside the static slot capacities (deg<=2 or rank overflow)."""
    atoms = np.asarray(atoms, dtype=np.float32)
    bonds = np.asarray(bonds, dtype=np.float32)
    edges = np.asarray(edges)
    W = np.asarray(W, dtype=np.float32)
    b = np.asarray(b, dtype=np.float32)

    deg = (edges != -1).sum(-1)                         # (B, A)
    out = np.zeros((B, A, C), dtype=np.float32)
    covered = np.zeros((B, A), dtype=bool)
    gi = np.arange(B) // G                              # global group index
    ii = np.arange(B) % G                               # molecule in group

    for d, cap, base in ((4, CAP4, 0), (3, CAP3, Q3_BASE)):
        mask = deg == d
        rank = np.cumsum(mask, axis=1)
        ok = mask & (rank <= cap)
        mi, ai = np.nonzero(ok)
        q = base + ii[mi] * cap + (rank[mi, ai] - 1)
        out[mi, ai] = out_hw[gi[mi], q].astype(np.float32)
        covered |= ok

    rest = (deg < D) & ~covered
    for m, a in zip(*np.nonzero(rest)):
        e = edges[m, a]
        e = e[e >= 0]
        fa = atoms[m, a] + (atoms[m, e].sum(0) if e.size else 0.0)
        feat = np.concatenate([fa, bonds[m, a].sum(0)])
        z = feat @ W[deg[m, a]] + b[deg[m, a]]
        out[m, a] = np.maximum(z, 0.0)
    return out


def run_sharded(atoms, bonds, edges, W, b, trace=False):
    """Run on the 8 NeuronCores; returns (output, BassKernelResults)."""
    from concourse.bass_utils import run_bass_kernel_spmd

    nc = _get_nc()
    in_maps = _make_in_maps(atoms, bonds, edges, W, b)
    res = run_bass_kernel_spmd(nc, in_maps, list(range(NCORES)), trace=trace)
    out_hw = np.concatenate(
        [np.asarray(res.results[c]["out"]) for c in range(NCORES)],
        axis=0)                                          # (NCORES*NG, Q, C)
    out = _assemble(out_hw, atoms, bonds, edges, W, b)
    return out, res


def kernel(atoms, bonds, edges, W, b):
    out, _ = run_sharded(atoms, bonds, edges, W, b)
    return out
